# revision 3
# baseline (speedup 1.0000x reference)
"""Trainium2 Bass kernel for nn_MicroBiMambaBackbone.

Sharding: pure data-parallel over batch (4 sequences per core x 8 cores).
Layout: channels on partitions, time on the free dimension.
Selective scan via DVE tensor_tensor_scan with s-major segment packing and
zero-decay boundary columns for cross-chunk state carry.
"""
import os
import sys

for _p in ("/opt/trn_rl_repo", "/root/.axon_site/_ro/trn_rl_repo"):
    if os.path.isdir(_p) and _p not in sys.path:
        sys.path.insert(0, _p)
os.environ.setdefault("MYCRO_LOCAL_CACHE", "1")

import numpy as np

import concourse.bass as bass
import concourse.bacc as bacc
import concourse.tile as tile
from concourse import mybir
from concourse.bass_utils import run_bass_kernel_spmd

F32 = mybir.dt.float32
AF = mybir.ActivationFunctionType
OP = mybir.AluOpType

# model dims
B, L, DIN = 32, 1024, 6
D, DI, S, K, DTR = 256, 512, 16, 4, 16
NL = 4
OUT = 128
NCORES = 8
NB = B // NCORES          # sequences per core
ND = D // 128             # d-tiles of model dim
NDI = DI // 128           # d-tiles of inner dim
TS = 512                  # time slab
NSLAB = L // TS
SG = 2                    # s-group size for scan ops
NSG = S // SG
EPS = 1e-5


def _ap(t, offset_delta, dims):
    return bass.AP(tensor=t.tensor, offset=t.offset + offset_delta, ap=dims)


def build(nb=NB, nlayers=NL, nslab=NSLAB, debug=False):
    nc = bacc.Bacc("TRN2", target_bir_lowering=False, debug=False)
    L_ = nslab * TS

    xf_d = nc.dram_tensor("xf", [nb, 6, L_], F32, kind="ExternalInput")
    pe2_d = nc.dram_tensor("pe2", [D, L_], F32, kind="ExternalInput")
    ddir_d = nc.dram_tensor("ddir", [D], F32, kind="ExternalInput")
    cwt_d = nc.dram_tensor("cont_wT", [5, D], F32, kind="ExternalInput")
    cb_d = nc.dram_tensor("cont_b", [D], F32, kind="ExternalInput")
    lng_d = nc.dram_tensor("ln_g", [D], F32, kind="ExternalInput")
    lnb_d = nc.dram_tensor("ln_b", [D], F32, kind="ExternalInput")
    inwt_d = nc.dram_tensor("in_wT", [nlayers, D, 2 * DI], F32, kind="ExternalInput")
    cvw_d = nc.dram_tensor("conv_w", [nlayers, DI, K], F32, kind="ExternalInput")
    cvb_d = nc.dram_tensor("conv_b", [nlayers, DI], F32, kind="ExternalInput")
    xpt_d = nc.dram_tensor("xproj_wT", [nlayers, DI, DTR + 2 * S], F32, kind="ExternalInput")
    dtwt_d = nc.dram_tensor("dt_wT", [nlayers, DTR, DI], F32, kind="ExternalInput")
    dtb_d = nc.dram_tensor("dt_b", [nlayers, DI], F32, kind="ExternalInput")
    A_d = nc.dram_tensor("A", [nlayers, DI, S], F32, kind="ExternalInput")
    Dp_d = nc.dram_tensor("Dp", [nlayers, DI], F32, kind="ExternalInput")
    owt_d = nc.dram_tensor("out_wT", [nlayers, DI, D], F32, kind="ExternalInput")
    ng_d = nc.dram_tensor("norm_g", [nlayers, D], F32, kind="ExternalInput")
    nb_d = nc.dram_tensor("norm_b", [nlayers, D], F32, kind="ExternalInput")
    pwt_d = nc.dram_tensor("proj_wT", [2 * D, OUT], F32, kind="ExternalInput")
    pb_d = nc.dram_tensor("proj_b", [OUT], F32, kind="ExternalInput")
    sel48_d = nc.dram_tensor("sel48", [DTR + 2 * S, 2 * S * 128], F32, kind="ExternalInput")
    sel6_d = nc.dram_tensor("sel6", [6, 128], F32, kind="ExternalInput")
    ones1_d = nc.dram_tensor("ones1", [1, 128], F32, kind="ExternalInput")

    out_d = nc.dram_tensor("out", [nb, OUT], F32, kind="ExternalOutput")
    dbg = {}
    if debug:
        for nm, sh in (("h0", [D, L_]), ("x1", [D, L_]), ("xi1", [DI, L_]),
                       ("dt1", [DI, L_]), ("y1", [DI, L_])):
            dbg[nm] = nc.dram_tensor("dbg_" + nm, sh, F32, kind="ExternalOutput")

    with tile.TileContext(nc) as tc:
        import contextlib
        with contextlib.ExitStack() as ctx:
            wpool = ctx.enter_context(tc.tile_pool(name="weights", bufs=1))
            wstr = ctx.enter_context(tc.tile_pool(name="wstream", bufs=1))
            apool = ctx.enter_context(tc.tile_pool(name="acts", bufs=1))
            spool = ctx.enter_context(tc.tile_pool(name="slab", bufs=1))
            s2pool = ctx.enter_context(tc.tile_pool(name="slab2", bufs=1))
            scpool = ctx.enter_context(tc.tile_pool(name="scan", bufs=1))
            rpool = ctx.enter_context(tc.tile_pool(name="rows", bufs=1))
            pp = ctx.enter_context(tc.tile_pool(name="ps_mm", bufs=2, space="PSUM"))
            pln = ctx.enter_context(tc.tile_pool(name="ps_ln", bufs=1, space="PSUM"))
            pbc = ctx.enter_context(tc.tile_pool(name="ps_bc", bufs=2, space="PSUM"))

            dma = nc.gpsimd.dma_start

            _wn = [0]

            def loadw(dram_ap, shape):
                _wn[0] += 1
                t = wpool.tile(shape, F32, name=f"w{_wn[0]}", tag=f"w{_wn[0]}")
                dma(out=t, in_=dram_ap)
                return t

            sel48 = loadw(sel48_d.ap(), [DTR + 2 * S, 2 * S * 128])
            sel6 = loadw(sel6_d.ap(), [6, 128])
            ones1 = loadw(ones1_d.ap(), [1, 128])
            onescol = wpool.tile([128, 1], F32)
            nc.vector.memset(onescol, 1.0)
            eps_t = wpool.tile([1, 1], F32)
            nc.vector.memset(eps_t, EPS)

            cwt = [loadw(cwt_d.ap()[:, m * 128:(m + 1) * 128], [5, 128]) for m in range(ND)]
            pe2 = loadw(pe2_d.ap().rearrange("(n p) l -> p n l", p=128), [128, ND, L_])

            def load_cols(dram_t, n, base):
                _wn[0] += 1
                t = wpool.tile([128, n], F32, name=f"w{_wn[0]}", tag=f"w{_wn[0]}")
                dma(out=t, in_=bass.AP(tensor=dram_t.ap().tensor, offset=base,
                                       ap=[[1, 128], [128, n]]))
                return t

            cont_b = load_cols(cb_d, ND, 0)
            ln_g = load_cols(lng_d, ND, 0)
            ln_b = load_cols(lnb_d, ND, 0)
            ddir = load_cols(ddir_d, ND, 0)
            pb_t = load_cols(pb_d, 1, 0)

            xpt = [[loadw(xpt_d.ap()[l, k * 128:(k + 1) * 128, :], [128, DTR + 2 * S])
                    for k in range(NDI)] for l in range(nlayers)]
            dtwt = [loadw(dtwt_d.ap()[l], [DTR, DI]) for l in range(nlayers)]
            owt = [[loadw(owt_d.ap()[l, k * 128:(k + 1) * 128, :], [128, D])
                    for k in range(NDI)] for l in range(nlayers)]
            pwt = [loadw(pwt_d.ap()[k * 128:(k + 1) * 128, :], [128, OUT])
                   for k in range(2 * ND)]

            def load_convw(l, m):
                _wn[0] += 1
                t = wpool.tile([128, K], F32, name=f"w{_wn[0]}", tag=f"w{_wn[0]}")
                dma(out=t, in_=bass.AP(tensor=cvw_d.ap().tensor,
                                       offset=(l * DI + m * 128) * K,
                                       ap=[[K, 128], [1, K]]))
                return t

            cvw = [[load_convw(l, m) for m in range(NDI)] for l in range(nlayers)]
            cvb = [load_cols(cvb_d, NDI, l * DI) for l in range(nlayers)]
            dtb = [load_cols(dtb_d, NDI, l * DI) for l in range(nlayers)]
            Dpw = [load_cols(Dp_d, NDI, l * DI) for l in range(nlayers)]
            ng = [load_cols(ng_d, ND, l * D) for l in range(nlayers)]
            nbt = [load_cols(nb_d, ND, l * D) for l in range(nlayers)]
            A_t = [[loadw(A_d.ap()[l, m * 128:(m + 1) * 128, :], [128, S])
                    for m in range(NDI)] for l in range(nlayers)]

            zcat = [apool.tile([128, nb], F32, tag=f"zcat{k}", name=f"zcat{k}") for k in range(2 * ND)]

            def layer_norm(x_aps, g_cols, b_cols, out_aps):
                ssum = pln.tile([1, TS], F32, tag="ln_sum")
                s2 = pln.tile([1, TS], F32, tag="ln_sum2")
                sqt = rpool.tile([128, TS], F32, tag="ln_sq")
                for i, xt in enumerate(x_aps):
                    nc.scalar.activation(out=sqt, in_=xt, func=AF.Square)
                    nc.tensor.matmul(s2, onescol, sqt,
                                     start=(i == 0), stop=(i == len(x_aps) - 1))
                for i, xt in enumerate(x_aps):
                    nc.tensor.matmul(ssum, onescol, xt,
                                     start=(i == 0), stop=(i == len(x_aps) - 1))
                murs = rpool.tile([1, 2 * TS], F32, tag="ln_murs")
                nc.scalar.activation(out=murs[:, 0:TS], in_=ssum, func=AF.Copy,
                                     scale=1.0 / D)
                r1 = rpool.tile([1, TS], F32, tag="ln_r1")
                nc.scalar.activation(out=r1, in_=s2, func=AF.Copy, scale=1.0 / D)
                r2 = rpool.tile([1, TS], F32, tag="ln_r2")
                nc.scalar.activation(out=r2, in_=murs[:, 0:TS], func=AF.Square)
                nc.vector.tensor_tensor(out=r1, in0=r1, in1=r2, op=OP.subtract)
                nc.scalar.activation(out=r1, in_=r1, func=AF.Ln, bias=eps_t[0:1, 0:1])
                nc.scalar.activation(out=murs[:, TS:], in_=r1, func=AF.Exp, scale=-0.5)
                lnbc = pln.tile([128, 2 * TS], F32, tag="ln_bc")
                nc.tensor.matmul(lnbc[:, 0:TS], ones1, murs[:, 0:TS],
                                 start=True, stop=True)
                nc.tensor.matmul(lnbc[:, TS:], ones1, murs[:, TS:],
                                 start=True, stop=True)
                t0v = rpool.tile([128, TS], F32, tag="ln_t0")
                for i, xt in enumerate(x_aps):
                    nc.vector.tensor_tensor(out=t0v, in0=xt, in1=lnbc[:, 0:TS],
                                            op=OP.subtract)
                    nc.vector.tensor_tensor(out=t0v, in0=t0v, in1=lnbc[:, TS:],
                                            op=OP.mult)
                    nc.vector.tensor_scalar(out=out_aps[i], in0=t0v,
                                            scalar1=g_cols[:, i:i + 1],
                                            scalar2=b_cols[:, i:i + 1],
                                            op0=OP.mult, op1=OP.add)

            for b in range(nb):
                # ===== embedding =====
                xf = apool.tile([6, L_], F32, tag="xf")
                dma(out=xf, in_=xf_d.ap()[b])
                h_fwd = apool.tile([128, ND, L_], F32, tag="h_fwd")
                h_rev = apool.tile([128, ND, L_], F32, tag="h_rev")
                for islab in range(nslab):
                    t0, t1 = islab * TS, (islab + 1) * TS
                    e_sb = spool.tile([128, ND, TS], F32, tag="emb_e")
                    for m in range(ND):
                        ep = pp.tile([128, TS], F32, tag="mm_ps")
                        nc.tensor.matmul(ep, cwt[m], xf[0:5, t0:t1], start=True, stop=True)
                        nc.scalar.activation(out=e_sb[:, m, :], in_=ep, func=AF.Identity,
                                             bias=cont_b[:, m:m + 1])
                    xn = spool.tile([128, ND, TS], F32, tag="xn")
                    layer_norm([e_sb[:, m, :] for m in range(ND)], ln_g, ln_b,
                               [xn[:, m, :] for m in range(ND)])
                    mb = pbc.tile([128, TS], F32, tag="bc_ps")
                    nc.tensor.matmul(mb, sel6, xf[:, t0:t1], start=True, stop=True)
                    for m in range(ND):
                        nc.scalar.activation(out=xn[:, m, :], in_=xn[:, m, :],
                                             func=AF.Gelu)
                        hm = h_fwd[:, m, t0:t1]
                        nc.vector.tensor_tensor(out=hm, in0=xn[:, m, :],
                                                in1=pe2[:, m, t0:t1], op=OP.add)
                        nc.vector.scalar_tensor_tensor(out=hm, in0=mb,
                                                       scalar=ddir[:, m:m + 1],
                                                       in1=hm, op0=OP.mult, op1=OP.add)
                for m in range(ND):
                    src = _ap(h_fwd, m * L_ + (L_ - 1), [h_fwd.ap[0], [-1, L_]])
                    nc.vector.tensor_copy(out=h_rev[:, m, :], in_=src)
                if debug and b == 0:
                    dma(out=dbg["h0"].ap().rearrange("(n p) l -> p n l", p=128), in_=h_fwd)

                # ===== mamba stacks =====
                for direction in range(2):
                    x_cur = h_fwd if direction == 0 else h_rev
                    lrange = (range(0, nlayers - nlayers // 2) if direction == 0
                              else range(nlayers - nlayers // 2, nlayers))
                    for li, l in enumerate(lrange):
                        inw = wstr.tile([128, ND, 2 * DI], F32, tag="inw")
                        dma(out=inw, in_=inwt_d.ap()[l].rearrange(
                            "(n p) e -> p n e", p=128))
                        if li == 0:
                            x_new = apool.tile([128, ND, L_], F32, tag="xnew0")
                        else:
                            x_new = h_fwd if direction == 0 else h_rev
                        carry = apool.tile([128, NDI, S], F32, tag="carry")
                        nc.vector.memset(carry, 0.0)
                        halo = apool.tile([128, NDI, K - 1], F32, tag="halo")
                        nc.vector.memset(halo, 0.0)
                        for islab in range(nslab):
                            t0, t1 = islab * TS, (islab + 1) * TS
                            xn = spool.tile([128, ND, TS], F32, tag="xn")
                            layer_norm([x_cur[:, m, t0:t1] for m in range(ND)],
                                       ng[l], nbt[l],
                                       [xn[:, m, :] for m in range(ND)])
                            xi_raw = spool.tile([128, NDI, K - 1 + TS], F32, tag="xi_raw")
                            z_t = spool.tile([128, NDI, TS], F32, tag="z")
                            xi_t = spool.tile([128, NDI, TS], F32, tag="xi")
                            dt_t = spool.tile([128, NDI, TS], F32, tag="dt")
                            y_t = spool.tile([128, NDI, TS], F32, tag="y")
                            nc.vector.tensor_copy(
                                out=_ap(xi_raw, 0,
                                        [xi_raw.ap[0], [K - 1 + TS, NDI], [1, K - 1]]),
                                in_=halo)
                            for m in range(2 * NDI):
                                psm = pp.tile([128, TS], F32, tag="mm_ps")
                                for k in range(ND):
                                    nc.tensor.matmul(psm, inw[:, k, m * 128:(m + 1) * 128],
                                                     xn[:, k, :], start=(k == 0),
                                                     stop=(k == ND - 1))
                                if m < NDI:
                                    nc.scalar.activation(out=xi_raw[:, m, K - 1:], in_=psm,
                                                         func=AF.Copy)
                                else:
                                    nc.scalar.activation(out=z_t[:, m - NDI, :],
                                                         in_=psm, func=AF.Copy)
                            nc.vector.tensor_copy(
                                out=halo,
                                in_=_ap(xi_raw, TS,
                                        [xi_raw.ap[0], [K - 1 + TS, NDI], [1, K - 1]]))
                            # conv + silu
                            for m in range(NDI):
                                acc = s2pool.tile([128, TS], F32, tag="convacc")
                                nc.vector.tensor_scalar(out=acc, in0=xi_raw[:, m, K - 1:],
                                                        scalar1=cvw[l][m][:, K - 1:K],
                                                        scalar2=None, op0=OP.mult)
                                for kk in range(K - 2, -1, -1):
                                    nc.vector.scalar_tensor_tensor(
                                        out=acc, in0=xi_raw[:, m, kk:kk + TS],
                                        scalar=cvw[l][m][:, kk:kk + 1],
                                        in1=acc, op0=OP.mult, op1=OP.add)
                                nc.scalar.activation(out=xi_t[:, m, :], in_=acc,
                                                     func=AF.Silu, bias=cvb[l][:, m:m + 1])
                                nc.scalar.activation(out=z_t[:, m, :], in_=z_t[:, m, :],
                                                     func=AF.Silu)
                            # xproj
                            xdb_ps = pp.tile([DTR + 2 * S, TS], F32, tag="mm_ps")
                            for k in range(NDI):
                                nc.tensor.matmul(xdb_ps, xpt[l][k], xi_t[:, k, :],
                                                 start=(k == 0), stop=(k == NDI - 1))
                            xdb = s2pool.tile([DTR + 2 * S, TS], F32, tag="xdb")
                            nc.scalar.activation(out=xdb, in_=xdb_ps, func=AF.Copy)
                            # dt proj + softplus; dtu
                            for m in range(NDI):
                                dps = pp.tile([128, TS], F32, tag="mm_ps")
                                nc.tensor.matmul(dps, dtwt[l][:, m * 128:(m + 1) * 128],
                                                 xdb[0:DTR, :], start=True, stop=True)
                                spx = s2pool.tile([128, TS], F32, tag="spx")
                                nc.scalar.activation(out=spx, in_=dps, func=AF.Exp,
                                                     bias=dtb[l][:, m:m + 1])
                                nc.scalar.activation(out=dt_t[:, m, :], in_=spx,
                                                     func=AF.Ln, bias=onescol[:, 0:1])
                                nc.vector.tensor_scalar(out=y_t[:, m, :],
                                                        in0=xi_t[:, m, :],
                                                        scalar1=Dpw[l][:, m:m + 1],
                                                        scalar2=None, op0=OP.mult)
                                nc.vector.tensor_tensor(out=xi_t[:, m, :],
                                                        in0=xi_t[:, m, :],
                                                        in1=dt_t[:, m, :], op=OP.mult)
                            # scan over s-groups
                            for g in range(NSG):
                                Bb = scpool.tile([128, SG, TS], F32, tag="Bb")
                                Cb = scpool.tile([128, SG, TS], F32, tag="Cb")
                                for j in range(SG):
                                    s = g * SG + j
                                    bp = pbc.tile([128, TS], F32, tag="bc_ps")
                                    nc.tensor.matmul(bp, sel48[:, s * 128:(s + 1) * 128],
                                                     xdb, start=True, stop=True)
                                    nc.scalar.activation(out=Bb[:, j, :], in_=bp,
                                                         func=AF.Copy)
                                    cp = pbc.tile([128, TS], F32, tag="bc_ps")
                                    nc.tensor.matmul(cp,
                                                     sel48[:, (S + s) * 128:(S + s + 1) * 128],
                                                     xdb, start=True, stop=True)
                                    nc.scalar.activation(out=Cb[:, j, :], in_=cp,
                                                         func=AF.Copy)
                                for m in range(NDI):
                                    a_t = scpool.tile([128, SG, TS + 1], F32, tag="a_t", bufs=2)
                                    b_t = scpool.tile([128, SG, TS + 1], F32, tag="b_t", bufs=2)
                                    h_t = scpool.tile([128, SG, TS + 1], F32, tag="h_t", bufs=2)
                                    for j in range(SG):
                                        s = g * SG + j
                                        nc.scalar.activation(out=a_t[:, j, 1:],
                                                             in_=dt_t[:, m, :],
                                                             func=AF.Exp,
                                                             scale=A_t[l][m][:, s:s + 1])
                                    nc.vector.memset(
                                        _ap(a_t, 0, [a_t.ap[0], [TS + 1, SG], [1, 1]]), 0.0)
                                    nc.vector.tensor_copy(
                                        out=_ap(b_t, 0, [b_t.ap[0], [TS + 1, SG], [1, 1]]),
                                        in_=_ap(carry, m * S + g * SG,
                                                [carry.ap[0], [1, SG], [1, 1]]))
                                    dtu_rep = _ap(xi_t, m * TS,
                                                  [xi_t.ap[0], [0, SG], [1, TS]])
                                    beng = nc.vector if m % 2 == 0 else nc.gpsimd
                                    beng.tensor_tensor(
                                        out=_ap(b_t, 1, [b_t.ap[0], [TS + 1, SG], [1, TS]]),
                                        in0=dtu_rep, in1=Bb, op=OP.mult)
                                    nc.vector.tensor_tensor_scan(
                                        out=_ap(h_t, 0, [h_t.ap[0], [1, SG * (TS + 1)]]),
                                        data0=_ap(a_t, 0, [a_t.ap[0], [1, SG * (TS + 1)]]),
                                        data1=_ap(b_t, 0, [b_t.ap[0], [1, SG * (TS + 1)]]),
                                        initial=0.0, op0=OP.mult, op1=OP.add)
                                    nc.vector.tensor_copy(
                                        out=_ap(carry, m * S + g * SG,
                                                [carry.ap[0], [1, SG], [1, 1]]),
                                        in_=_ap(h_t, TS, [h_t.ap[0], [TS + 1, SG], [1, 1]]))
                                    p_t = scpool.tile([128, SG, TS], F32, tag="p_t",
                                                      bufs=2)
                                    nc.gpsimd.tensor_tensor(
                                        out=p_t,
                                        in0=_ap(h_t, 1, [h_t.ap[0], [TS + 1, SG], [1, TS]]),
                                        in1=Cb, op=OP.mult)
                                    yg = s2pool.tile([128, TS], F32, tag="yg")
                                    nc.vector.tensor_tensor(out=yg, in0=p_t[:, 0, :],
                                                            in1=p_t[:, 1, :], op=OP.add)
                                    nc.vector.tensor_tensor(out=y_t[:, m, :],
                                                            in0=y_t[:, m, :],
                                                            in1=yg, op=OP.add)
                            # gate (z already silu'd at evac)
                            for m in range(NDI):
                                nc.vector.tensor_tensor(out=y_t[:, m, :], in0=y_t[:, m, :],
                                                        in1=z_t[:, m, :], op=OP.mult)
                            # out_proj + residual
                            for m in range(ND):
                                ops = pp.tile([128, TS], F32, tag="mm_ps")
                                for k in range(NDI):
                                    nc.tensor.matmul(ops, owt[l][k][:, m * 128:(m + 1) * 128],
                                                     y_t[:, k, :], start=(k == 0),
                                                     stop=(k == NDI - 1))
                                nc.vector.tensor_tensor(out=x_new[:, m, t0:t1],
                                                        in0=x_cur[:, m, t0:t1],
                                                        in1=ops, op=OP.add)
                            if debug and b == 0 and l == 0:
                                for m in range(NDI):
                                    dma(out=dbg["xi1"].ap().rearrange(
                                        "(n p) l -> p n l", p=128)[:, m, t0:t1],
                                        in_=xi_t[:, m, :])
                                    dma(out=dbg["dt1"].ap().rearrange(
                                        "(n p) l -> p n l", p=128)[:, m, t0:t1],
                                        in_=dt_t[:, m, :])
                                    dma(out=dbg["y1"].ap().rearrange(
                                        "(n p) l -> p n l", p=128)[:, m, t0:t1],
                                        in_=y_t[:, m, :])
                        x_cur = x_new
                        if debug and b == 0 and l == 0:
                            dma(out=dbg["x1"].ap().rearrange("(n p) l -> p n l", p=128),
                                in_=x_cur)
                    for m in range(ND):
                        mean = rpool.tile([128, 1], F32, tag="mean")
                        nc.vector.tensor_reduce(out=mean, in_=x_cur[:, m, :],
                                                axis=mybir.AxisListType.X, op=OP.add)
                        nc.scalar.activation(out=zcat[direction * ND + m][:, b:b + 1],
                                             in_=mean, func=AF.Copy, scale=1.0 / L_)

            prj = pp.tile([OUT, nb], F32, tag="mm_ps")
            for k in range(2 * ND):
                nc.tensor.matmul(prj, pwt[k], zcat[k], start=(k == 0),
                                 stop=(k == 2 * ND - 1))
            ob = rpool.tile([OUT, nb], F32, tag="out_sb")
            nc.scalar.activation(out=ob, in_=prj, func=AF.Identity, bias=pb_t[:, 0:1])
            dma(out=bass.AP(tensor=out_d.ap().tensor, offset=0,
                            ap=[[1, OUT], [OUT, nb]]), in_=ob)
    nc.compile()
    return nc


_cache = {}


def _prep_common(inputs, nlayers=NL, L_=L):
    import math
    pos = np.arange(L_, dtype=np.float32)[:, None]
    div = np.exp(np.arange(0, D, 2, dtype=np.float32) * (-math.log(10000.0) / D))
    pe = np.zeros((L_, D), np.float32)
    pe[:, 0::2] = np.sin(pos * div)
    pe[:, 1::2] = np.cos(pos * div)
    dir_emb = np.asarray(inputs["dir_emb"], np.float32)
    pe2 = np.ascontiguousarray((pe + dir_emb[0][None, :]).T)

    common = dict(
        pe2=pe2,
        ddir=np.ascontiguousarray(dir_emb[1] - dir_emb[0]),
        cont_wT=np.ascontiguousarray(np.asarray(inputs["cont_w"], np.float32).T),
        cont_b=np.asarray(inputs["cont_b"], np.float32),
        ln_g=np.asarray(inputs["ln_g"], np.float32),
        ln_b=np.asarray(inputs["ln_b"], np.float32),
        in_wT=np.ascontiguousarray(
            np.asarray(inputs["in_w"], np.float32)[:nlayers].transpose(0, 2, 1)),
        conv_w=np.ascontiguousarray(
            np.asarray(inputs["conv_w"], np.float32)[:nlayers, :, 0, :]),
        conv_b=np.asarray(inputs["conv_b"], np.float32)[:nlayers],
        xproj_wT=np.ascontiguousarray(
            np.asarray(inputs["xproj_w"], np.float32)[:nlayers].transpose(0, 2, 1)),
        dt_wT=np.ascontiguousarray(
            np.asarray(inputs["dt_w"], np.float32)[:nlayers].transpose(0, 2, 1)),
        dt_b=np.asarray(inputs["dt_b"], np.float32)[:nlayers],
        A=np.ascontiguousarray(
            -np.exp(np.asarray(inputs["A_log"], np.float32)[:nlayers])),
        Dp=np.asarray(inputs["Dp"], np.float32)[:nlayers],
        out_wT=np.ascontiguousarray(
            np.asarray(inputs["out_w"], np.float32)[:nlayers].transpose(0, 2, 1)),
        norm_g=np.asarray(inputs["norm_g"], np.float32)[:nlayers],
        norm_b=np.asarray(inputs["norm_b"], np.float32)[:nlayers],
        proj_wT=np.ascontiguousarray(np.asarray(inputs["proj_w"], np.float32).T),
        proj_b=np.asarray(inputs["proj_b"], np.float32),
    )
    sel48 = np.zeros((DTR + 2 * S, 2 * S * 128), np.float32)
    for i in range(2 * S):
        sel48[DTR + i, i * 128:(i + 1) * 128] = 1.0
    sel6 = np.zeros((6, 128), np.float32)
    sel6[5, :] = 1.0
    common["sel48"] = sel48
    common["sel6"] = sel6
    common["ones1"] = np.ones((1, 128), np.float32)
    return common


def _prep_xf(inputs, L_=L):
    x = np.asarray(inputs["x"], np.float32)
    cont_idx = [0, 1, 3, 4, 5]
    xs = x[:, :L_]
    xf = np.empty((B, 6, L_), np.float32)
    xf[:, 0:5, :] = xs[..., cont_idx].transpose(0, 2, 1)
    xf[:, 5, :] = (xs[:, :, 2] > 0).astype(np.float32)
    return xf


# weight-bearing inputs whose values feed _prep_common (everything but x)
_WKEYS = ("cont_w", "cont_b", "ln_g", "ln_b", "dir_emb", "in_w", "conv_w",
          "conv_b", "xproj_w", "dt_w", "dt_b", "A_log", "Dp", "out_w",
          "norm_g", "norm_b", "proj_w", "proj_b")


def _make_runner(nc, ncores=NCORES):
    """Persistent jit over the bass module, mirroring bass2jax.run_bass_via_pjrt
    but built once so steady-state calls skip retrace/recompile/NEFF reload."""
    import jax
    from jax.experimental.shard_map import shard_map
    from jax.sharding import Mesh, PartitionSpec, NamedSharding
    from concourse import bass2jax
    from concourse.bass2jax import _bass_exec_p, partition_id_tensor

    bass2jax.install_neuronx_cc_hook()

    partition_name = nc.partition_id_tensor.name if nc.partition_id_tensor else None
    in_names, out_names, out_avals, zero_shapes = [], [], [], []
    for alloc in nc.m.functions[0].allocations:
        if not isinstance(alloc, mybir.MemoryLocationSet):
            continue
        name = alloc.memorylocations[0].name
        if alloc.kind == "ExternalInput":
            if name != partition_name:
                in_names.append(name)
        elif alloc.kind == "ExternalOutput":
            out_names.append(name)
            shape = tuple(alloc.tensor_shape)
            dtype = mybir.dt.np(alloc.dtype)
            out_avals.append(jax.core.ShapedArray(shape, dtype))
            zero_shapes.append(((ncores * shape[0], *shape[1:]), dtype))
    n_params = len(in_names)
    n_outs = len(out_names)
    all_in = list(in_names) + list(out_names)
    if partition_name is not None:
        all_in.append(partition_name)

    def _body(*args):
        operands = list(args)
        if partition_name is not None:
            operands.append(partition_id_tensor())
        outs = _bass_exec_p.bind(
            *operands,
            out_avals=tuple(out_avals),
            in_names=tuple(all_in),
            out_names=tuple(out_names),
            lowering_input_output_aliases=(),
            sim_require_finite=True,
            sim_require_nnan=True,
            nc=nc,
        )
        return tuple(outs)

    devices = jax.devices()[:ncores]
    mesh = Mesh(np.asarray(devices), ("core",))
    P = PartitionSpec
    jitfn = jax.jit(
        shard_map(_body, mesh=mesh,
                  in_specs=(P("core"),) * (n_params + n_outs),
                  out_specs=(P("core"),) * n_outs,
                  check_rep=False),
        donate_argnums=tuple(range(n_params, n_params + n_outs)),
        keep_unused=True)
    sharding = NamedSharding(mesh, P("core"))
    return dict(jitfn=jitfn, in_names=in_names, out_names=out_names,
                zero_shapes=zero_shapes, sharding=sharding, jax=jax)


def _weights_current(inputs):
    cached = _cache.get("wraw")
    if cached is None:
        return False
    for k in _WKEYS:
        a, b = cached[k], inputs[k]
        if a is b:
            continue
        if not np.array_equal(a, np.asarray(b)):
            return False
    return True


def kernel(**inputs):
    if bool(int(os.environ.get("KERNEL_TRACE", "0"))):
        return _kernel_traced(**inputs)
    if "nc" not in _cache:
        _cache["nc"] = build()
        _cache["runner"] = _make_runner(_cache["nc"])
    run = _cache["runner"]
    jax = run["jax"]

    if not _weights_current(inputs):
        common = _prep_common(inputs)
        dev = {}
        for name in run["in_names"]:
            if name == "xf":
                continue
            w = common[name]
            rep = np.concatenate([w] * NCORES, axis=0)
            dev[name] = jax.device_put(rep, run["sharding"])
        _cache["wdev"] = dev
        _cache["wraw"] = {k: np.asarray(inputs[k]) for k in _WKEYS}

    xf = _prep_xf(inputs)
    wdev = _cache["wdev"]
    args = [xf if name == "xf" else wdev[name] for name in run["in_names"]]
    args += [np.zeros(sh, dt) for sh, dt in run["zero_shapes"]]
    outs = run["jitfn"](*args)
    oidx = run["out_names"].index("out")
    out = np.asarray(outs[oidx])          # (B, OUT), batch-concat across cores
    return np.ascontiguousarray(out.astype(np.float32))


def _kernel_traced(**inputs):
    """Profiling path: one-shot run through run_bass_kernel_spmd with trace."""
    if "nc" not in _cache:
        _cache["nc"] = build()
    nc = _cache["nc"]
    common = _prep_common(inputs)
    xf = _prep_xf(inputs)
    in_maps = []
    for c in range(NCORES):
        m = dict(common)
        m["xf"] = np.ascontiguousarray(xf[c * NB:(c + 1) * NB])
        in_maps.append(m)
    res = run_bass_kernel_spmd(nc, in_maps, core_ids=list(range(NCORES)),
                               trace=True)
    _cache["last_result"] = res
    out = np.concatenate([res.results[c]["out"] for c in range(NCORES)], axis=0)
    return np.ascontiguousarray(out.astype(np.float32))



# revision 7
# speedup vs baseline: 1.2038x; 1.2038x over previous
"""Trainium2 Bass kernel for nn_MicroBiMambaBackbone.

Sharding: pure data-parallel over batch (4 sequences per core x 8 cores).
Layout: channels on partitions, time on the free dimension.
Selective scan via DVE tensor_tensor_scan with s-major segment packing and
zero-decay boundary columns for cross-chunk state carry.
"""
import os
import sys

for _p in ("/opt/trn_rl_repo", "/root/.axon_site/_ro/trn_rl_repo"):
    if os.path.isdir(_p) and _p not in sys.path:
        sys.path.insert(0, _p)
os.environ.setdefault("MYCRO_LOCAL_CACHE", "1")

import numpy as np

import concourse.bass as bass
import concourse.bacc as bacc
import concourse.tile as tile
from concourse import mybir
from concourse.bass_utils import run_bass_kernel_spmd

F32 = mybir.dt.float32
F16 = mybir.dt.float16
AF = mybir.ActivationFunctionType
OP = mybir.AluOpType

# model dims
B, L, DIN = 32, 1024, 6
D, DI, S, K, DTR = 256, 512, 16, 4, 16
NL = 4
OUT = 128
NCORES = 8
NB = B // NCORES          # sequences per core
ND = D // 128             # d-tiles of model dim
NDI = DI // 128           # d-tiles of inner dim
TS = 512                  # time slab
NSLAB = L // TS
SG = 2                    # s-group size for scan ops
NSG = S // SG
EPS = 1e-5


def _ap(t, offset_delta, dims):
    return bass.AP(tensor=t.tensor, offset=t.offset + offset_delta, ap=dims)


def build(nb=NB, nlayers=NL, nslab=NSLAB, debug=False):
    nc = bacc.Bacc("TRN2", target_bir_lowering=False, debug=False)
    L_ = nslab * TS

    xf_d = nc.dram_tensor("xf", [nb, 6, L_], F16, kind="ExternalInput")
    pe2_d = nc.dram_tensor("pe2", [D, L_], F32, kind="ExternalInput")
    ddir_d = nc.dram_tensor("ddir", [D], F32, kind="ExternalInput")
    cwt_d = nc.dram_tensor("cont_wT", [5, D], F32, kind="ExternalInput")
    cb_d = nc.dram_tensor("cont_b", [D], F32, kind="ExternalInput")
    lng_d = nc.dram_tensor("ln_g", [D], F32, kind="ExternalInput")
    lnb_d = nc.dram_tensor("ln_b", [D], F32, kind="ExternalInput")
    inwt_d = nc.dram_tensor("in_wT", [nlayers, D, 2 * DI], F32, kind="ExternalInput")
    cvw_d = nc.dram_tensor("conv_w", [nlayers, DI, K], F32, kind="ExternalInput")
    cvb_d = nc.dram_tensor("conv_b", [nlayers, DI], F32, kind="ExternalInput")
    xpt_d = nc.dram_tensor("xproj_wT", [nlayers, DI, DTR + 2 * S], F32, kind="ExternalInput")
    dtwt_d = nc.dram_tensor("dt_wT", [nlayers, DTR, DI], F32, kind="ExternalInput")
    dtb_d = nc.dram_tensor("dt_b", [nlayers, DI], F32, kind="ExternalInput")
    A_d = nc.dram_tensor("A", [nlayers, DI, S], F32, kind="ExternalInput")
    Dp_d = nc.dram_tensor("Dp", [nlayers, DI], F32, kind="ExternalInput")
    owt_d = nc.dram_tensor("out_wT", [nlayers, DI, D], F32, kind="ExternalInput")
    ng_d = nc.dram_tensor("norm_g", [nlayers, D], F32, kind="ExternalInput")
    nb_d = nc.dram_tensor("norm_b", [nlayers, D], F32, kind="ExternalInput")
    pwt_d = nc.dram_tensor("proj_wT", [2 * D, OUT], F32, kind="ExternalInput")
    pb_d = nc.dram_tensor("proj_b", [OUT], F32, kind="ExternalInput")
    sel48_d = nc.dram_tensor("sel48", [DTR + 2 * S, 2 * S * 128], F32, kind="ExternalInput")
    sel6_d = nc.dram_tensor("sel6", [6, 128], F32, kind="ExternalInput")
    ones1_d = nc.dram_tensor("ones1", [1, 128], F32, kind="ExternalInput")

    out_d = nc.dram_tensor("out", [nb, OUT], F32, kind="ExternalOutput")
    dbg = {}
    if debug:
        for nm, sh in (("h0", [D, L_]), ("x1", [D, L_]), ("xi1", [DI, L_]),
                       ("dt1", [DI, L_]), ("y1", [DI, L_])):
            dbg[nm] = nc.dram_tensor("dbg_" + nm, sh, F32, kind="ExternalOutput")

    with tile.TileContext(nc) as tc:
        import contextlib
        with contextlib.ExitStack() as ctx:
            wpool = ctx.enter_context(tc.tile_pool(name="weights", bufs=1))
            wstr = ctx.enter_context(tc.tile_pool(name="wstream", bufs=1))
            apool = ctx.enter_context(tc.tile_pool(name="acts", bufs=1))
            spool = ctx.enter_context(tc.tile_pool(name="slab", bufs=1))
            s2pool = ctx.enter_context(tc.tile_pool(name="slab2", bufs=1))
            scpool = ctx.enter_context(tc.tile_pool(name="scan", bufs=1))
            rpool = ctx.enter_context(tc.tile_pool(name="rows", bufs=1))
            pp = ctx.enter_context(tc.tile_pool(name="ps_mm", bufs=2, space="PSUM"))
            pln = ctx.enter_context(tc.tile_pool(name="ps_ln", bufs=1, space="PSUM"))
            pbc = ctx.enter_context(tc.tile_pool(name="ps_bc", bufs=2, space="PSUM"))

            dma = nc.gpsimd.dma_start

            _wn = [0]

            def loadw(dram_ap, shape):
                _wn[0] += 1
                t = wpool.tile(shape, F32, name=f"w{_wn[0]}", tag=f"w{_wn[0]}")
                dma(out=t, in_=dram_ap)
                return t

            sel48 = loadw(sel48_d.ap(), [DTR + 2 * S, 2 * S * 128])
            sel6 = loadw(sel6_d.ap(), [6, 128])
            ones1 = loadw(ones1_d.ap(), [1, 128])
            onescol = wpool.tile([128, 1], F32)
            nc.vector.memset(onescol, 1.0)
            eps_t = wpool.tile([1, 1], F32)
            nc.vector.memset(eps_t, EPS)

            cwt = [loadw(cwt_d.ap()[:, m * 128:(m + 1) * 128], [5, 128]) for m in range(ND)]
            pe2 = loadw(pe2_d.ap().rearrange("(n p) l -> p n l", p=128), [128, ND, L_])

            def load_cols(dram_t, n, base):
                _wn[0] += 1
                t = wpool.tile([128, n], F32, name=f"w{_wn[0]}", tag=f"w{_wn[0]}")
                dma(out=t, in_=bass.AP(tensor=dram_t.ap().tensor, offset=base,
                                       ap=[[1, 128], [128, n]]))
                return t

            cont_b = load_cols(cb_d, ND, 0)
            ln_g = load_cols(lng_d, ND, 0)
            ln_b = load_cols(lnb_d, ND, 0)
            ddir = load_cols(ddir_d, ND, 0)
            pb_t = load_cols(pb_d, 1, 0)

            xpt = [[loadw(xpt_d.ap()[l, k * 128:(k + 1) * 128, :], [128, DTR + 2 * S])
                    for k in range(NDI)] for l in range(nlayers)]
            dtwt = [loadw(dtwt_d.ap()[l], [DTR, DI]) for l in range(nlayers)]
            owt = [[loadw(owt_d.ap()[l, k * 128:(k + 1) * 128, :], [128, D])
                    for k in range(NDI)] for l in range(nlayers)]
            pwt = [loadw(pwt_d.ap()[k * 128:(k + 1) * 128, :], [128, OUT])
                   for k in range(2 * ND)]

            def load_convw(l, m):
                _wn[0] += 1
                t = wpool.tile([128, K], F32, name=f"w{_wn[0]}", tag=f"w{_wn[0]}")
                dma(out=t, in_=bass.AP(tensor=cvw_d.ap().tensor,
                                       offset=(l * DI + m * 128) * K,
                                       ap=[[K, 128], [1, K]]))
                return t

            cvw = [[load_convw(l, m) for m in range(NDI)] for l in range(nlayers)]
            cvb = [load_cols(cvb_d, NDI, l * DI) for l in range(nlayers)]
            dtb = [load_cols(dtb_d, NDI, l * DI) for l in range(nlayers)]
            Dpw = [load_cols(Dp_d, NDI, l * DI) for l in range(nlayers)]
            ng = [load_cols(ng_d, ND, l * D) for l in range(nlayers)]
            nbt = [load_cols(nb_d, ND, l * D) for l in range(nlayers)]
            A_t = [[loadw(A_d.ap()[l, m * 128:(m + 1) * 128, :], [128, S])
                    for m in range(NDI)] for l in range(nlayers)]

            zcat = [apool.tile([128, nb], F32, tag=f"zcat{k}", name=f"zcat{k}") for k in range(2 * ND)]

            def layer_norm(x_aps, g_cols, b_cols, out_aps):
                ssum = pln.tile([1, TS], F32, tag="ln_sum")
                s2 = pln.tile([1, TS], F32, tag="ln_sum2")
                sqt = rpool.tile([128, TS], F32, tag="ln_sq")
                for i, xt in enumerate(x_aps):
                    nc.scalar.activation(out=sqt, in_=xt, func=AF.Square)
                    nc.tensor.matmul(s2, onescol, sqt,
                                     start=(i == 0), stop=(i == len(x_aps) - 1))
                for i, xt in enumerate(x_aps):
                    nc.tensor.matmul(ssum, onescol, xt,
                                     start=(i == 0), stop=(i == len(x_aps) - 1))
                murs = rpool.tile([1, 2 * TS], F32, tag="ln_murs")
                nc.scalar.activation(out=murs[:, 0:TS], in_=ssum, func=AF.Copy,
                                     scale=1.0 / D)
                r1 = rpool.tile([1, TS], F32, tag="ln_r1")
                nc.scalar.activation(out=r1, in_=s2, func=AF.Copy, scale=1.0 / D)
                r2 = rpool.tile([1, TS], F32, tag="ln_r2")
                nc.scalar.activation(out=r2, in_=murs[:, 0:TS], func=AF.Square)
                nc.vector.tensor_tensor(out=r1, in0=r1, in1=r2, op=OP.subtract)
                nc.scalar.activation(out=r1, in_=r1, func=AF.Ln, bias=eps_t[0:1, 0:1])
                nc.scalar.activation(out=murs[:, TS:], in_=r1, func=AF.Exp, scale=-0.5)
                lnbc = pln.tile([128, 2 * TS], F32, tag="ln_bc")
                nc.tensor.matmul(lnbc[:, 0:TS], ones1, murs[:, 0:TS],
                                 start=True, stop=True)
                nc.tensor.matmul(lnbc[:, TS:], ones1, murs[:, TS:],
                                 start=True, stop=True)
                t0v = rpool.tile([128, TS], F32, tag="ln_t0")
                for i, xt in enumerate(x_aps):
                    nc.vector.tensor_tensor(out=t0v, in0=xt, in1=lnbc[:, 0:TS],
                                            op=OP.subtract)
                    nc.vector.tensor_tensor(out=t0v, in0=t0v, in1=lnbc[:, TS:],
                                            op=OP.mult)
                    nc.vector.tensor_scalar(out=out_aps[i], in0=t0v,
                                            scalar1=g_cols[:, i:i + 1],
                                            scalar2=b_cols[:, i:i + 1],
                                            op0=OP.mult, op1=OP.add)

            for b in range(nb):
                # ===== embedding =====
                xf16 = apool.tile([6, L_], F16, tag="xf16")
                dma(out=xf16, in_=xf_d.ap()[b])
                xf = apool.tile([6, L_], F32, tag="xf")
                nc.vector.tensor_copy(out=xf, in_=xf16)
                h_fwd = apool.tile([128, ND, L_], F32, tag="h_fwd")
                h_rev = apool.tile([128, ND, L_], F32, tag="h_rev")
                for islab in range(nslab):
                    t0, t1 = islab * TS, (islab + 1) * TS
                    e_sb = spool.tile([128, ND, TS], F32, tag="emb_e")
                    for m in range(ND):
                        ep = pp.tile([128, TS], F32, tag="mm_ps")
                        nc.tensor.matmul(ep, cwt[m], xf[0:5, t0:t1], start=True, stop=True)
                        nc.scalar.activation(out=e_sb[:, m, :], in_=ep, func=AF.Identity,
                                             bias=cont_b[:, m:m + 1])
                    xn = spool.tile([128, ND, TS], F32, tag="xn")
                    layer_norm([e_sb[:, m, :] for m in range(ND)], ln_g, ln_b,
                               [xn[:, m, :] for m in range(ND)])
                    mb = pbc.tile([128, TS], F32, tag="bc_ps")
                    nc.tensor.matmul(mb, sel6, xf[:, t0:t1], start=True, stop=True)
                    for m in range(ND):
                        nc.scalar.activation(out=xn[:, m, :], in_=xn[:, m, :],
                                             func=AF.Gelu)
                        hm = h_fwd[:, m, t0:t1]
                        nc.vector.tensor_tensor(out=hm, in0=xn[:, m, :],
                                                in1=pe2[:, m, t0:t1], op=OP.add)
                        nc.vector.scalar_tensor_tensor(out=hm, in0=mb,
                                                       scalar=ddir[:, m:m + 1],
                                                       in1=hm, op0=OP.mult, op1=OP.add)
                for m in range(ND):
                    src = _ap(h_fwd, m * L_ + (L_ - 1), [h_fwd.ap[0], [-1, L_]])
                    nc.vector.tensor_copy(out=h_rev[:, m, :], in_=src)
                if debug and b == 0:
                    dma(out=dbg["h0"].ap().rearrange("(n p) l -> p n l", p=128), in_=h_fwd)

                # ===== mamba stacks =====
                for direction in range(2):
                    x_cur = h_fwd if direction == 0 else h_rev
                    lrange = (range(0, nlayers - nlayers // 2) if direction == 0
                              else range(nlayers - nlayers // 2, nlayers))
                    for li, l in enumerate(lrange):
                        inw = wstr.tile([128, ND, 2 * DI], F32, tag="inw")
                        dma(out=inw, in_=inwt_d.ap()[l].rearrange(
                            "(n p) e -> p n e", p=128))
                        if li == 0:
                            x_new = apool.tile([128, ND, L_], F32, tag="xnew0")
                        else:
                            x_new = h_fwd if direction == 0 else h_rev
                        carry = apool.tile([128, NDI, S], F32, tag="carry")
                        nc.vector.memset(carry, 0.0)
                        halo = apool.tile([128, NDI, K - 1], F32, tag="halo")
                        nc.vector.memset(halo, 0.0)
                        for islab in range(nslab):
                            t0, t1 = islab * TS, (islab + 1) * TS
                            xn = spool.tile([128, ND, TS], F32, tag="xn")
                            layer_norm([x_cur[:, m, t0:t1] for m in range(ND)],
                                       ng[l], nbt[l],
                                       [xn[:, m, :] for m in range(ND)])
                            xi_raw = spool.tile([128, NDI, K - 1 + TS], F32, tag="xi_raw")
                            z_t = spool.tile([128, NDI, TS], F32, tag="z")
                            xi_t = spool.tile([128, NDI, TS], F32, tag="xi")
                            dt_t = spool.tile([128, NDI, TS], F32, tag="dt")
                            y_t = spool.tile([128, NDI, TS], F32, tag="y")
                            nc.vector.tensor_copy(
                                out=_ap(xi_raw, 0,
                                        [xi_raw.ap[0], [K - 1 + TS, NDI], [1, K - 1]]),
                                in_=halo)
                            for m in range(2 * NDI):
                                psm = pp.tile([128, TS], F32, tag="mm_ps")
                                for k in range(ND):
                                    nc.tensor.matmul(psm, inw[:, k, m * 128:(m + 1) * 128],
                                                     xn[:, k, :], start=(k == 0),
                                                     stop=(k == ND - 1))
                                if m < NDI:
                                    nc.scalar.activation(out=xi_raw[:, m, K - 1:], in_=psm,
                                                         func=AF.Copy)
                                else:
                                    nc.scalar.activation(out=z_t[:, m - NDI, :],
                                                         in_=psm, func=AF.Copy)
                            nc.vector.tensor_copy(
                                out=halo,
                                in_=_ap(xi_raw, TS,
                                        [xi_raw.ap[0], [K - 1 + TS, NDI], [1, K - 1]]))
                            # conv + silu
                            for m in range(NDI):
                                acc = s2pool.tile([128, TS], F32, tag="convacc")
                                nc.vector.tensor_scalar(out=acc, in0=xi_raw[:, m, K - 1:],
                                                        scalar1=cvw[l][m][:, K - 1:K],
                                                        scalar2=None, op0=OP.mult)
                                for kk in range(K - 2, -1, -1):
                                    nc.vector.scalar_tensor_tensor(
                                        out=acc, in0=xi_raw[:, m, kk:kk + TS],
                                        scalar=cvw[l][m][:, kk:kk + 1],
                                        in1=acc, op0=OP.mult, op1=OP.add)
                                nc.scalar.activation(out=xi_t[:, m, :], in_=acc,
                                                     func=AF.Silu, bias=cvb[l][:, m:m + 1])
                                nc.scalar.activation(out=z_t[:, m, :], in_=z_t[:, m, :],
                                                     func=AF.Silu)
                            # xproj
                            xdb_ps = pp.tile([DTR + 2 * S, TS], F32, tag="mm_ps")
                            for k in range(NDI):
                                nc.tensor.matmul(xdb_ps, xpt[l][k], xi_t[:, k, :],
                                                 start=(k == 0), stop=(k == NDI - 1))
                            xdb = s2pool.tile([DTR + 2 * S, TS], F32, tag="xdb")
                            nc.scalar.activation(out=xdb, in_=xdb_ps, func=AF.Copy)
                            # dt proj + softplus; dtu
                            for m in range(NDI):
                                dps = pp.tile([128, TS], F32, tag="mm_ps")
                                nc.tensor.matmul(dps, dtwt[l][:, m * 128:(m + 1) * 128],
                                                 xdb[0:DTR, :], start=True, stop=True)
                                spx = s2pool.tile([128, TS], F32, tag="spx")
                                nc.scalar.activation(out=spx, in_=dps, func=AF.Exp,
                                                     bias=dtb[l][:, m:m + 1])
                                nc.scalar.activation(out=dt_t[:, m, :], in_=spx,
                                                     func=AF.Ln, bias=onescol[:, 0:1])
                                nc.vector.tensor_scalar(out=y_t[:, m, :],
                                                        in0=xi_t[:, m, :],
                                                        scalar1=Dpw[l][:, m:m + 1],
                                                        scalar2=None, op0=OP.mult)
                                nc.vector.tensor_tensor(out=xi_t[:, m, :],
                                                        in0=xi_t[:, m, :],
                                                        in1=dt_t[:, m, :], op=OP.mult)
                            # scan over s-groups
                            for g in range(NSG):
                                Bb = scpool.tile([128, SG, TS], F32, tag="Bb")
                                Cb = scpool.tile([128, SG, TS], F32, tag="Cb")
                                for j in range(SG):
                                    s = g * SG + j
                                    bp = pbc.tile([128, TS], F32, tag="bc_ps")
                                    nc.tensor.matmul(bp, sel48[:, s * 128:(s + 1) * 128],
                                                     xdb, start=True, stop=True)
                                    nc.scalar.activation(out=Bb[:, j, :], in_=bp,
                                                         func=AF.Copy)
                                    cp = pbc.tile([128, TS], F32, tag="bc_ps")
                                    nc.tensor.matmul(cp,
                                                     sel48[:, (S + s) * 128:(S + s + 1) * 128],
                                                     xdb, start=True, stop=True)
                                    nc.scalar.activation(out=Cb[:, j, :], in_=cp,
                                                         func=AF.Copy)
                                for m in range(NDI):
                                    a_t = scpool.tile([128, SG, TS + 1], F32, tag="a_t", bufs=2)
                                    b_t = scpool.tile([128, SG, TS + 1], F32, tag="b_t", bufs=2)
                                    h_t = scpool.tile([128, SG, TS + 1], F32, tag="h_t", bufs=2)
                                    for j in range(SG):
                                        s = g * SG + j
                                        nc.scalar.activation(out=a_t[:, j, 1:],
                                                             in_=dt_t[:, m, :],
                                                             func=AF.Exp,
                                                             scale=A_t[l][m][:, s:s + 1])
                                    nc.vector.memset(
                                        _ap(a_t, 0, [a_t.ap[0], [TS + 1, SG], [1, 1]]), 0.0)
                                    nc.vector.tensor_copy(
                                        out=_ap(b_t, 0, [b_t.ap[0], [TS + 1, SG], [1, 1]]),
                                        in_=_ap(carry, m * S + g * SG,
                                                [carry.ap[0], [1, SG], [1, 1]]))
                                    dtu_rep = _ap(xi_t, m * TS,
                                                  [xi_t.ap[0], [0, SG], [1, TS]])
                                    beng = nc.vector if m % 2 == 0 else nc.gpsimd
                                    beng.tensor_tensor(
                                        out=_ap(b_t, 1, [b_t.ap[0], [TS + 1, SG], [1, TS]]),
                                        in0=dtu_rep, in1=Bb, op=OP.mult)
                                    nc.vector.tensor_tensor_scan(
                                        out=_ap(h_t, 0, [h_t.ap[0], [1, SG * (TS + 1)]]),
                                        data0=_ap(a_t, 0, [a_t.ap[0], [1, SG * (TS + 1)]]),
                                        data1=_ap(b_t, 0, [b_t.ap[0], [1, SG * (TS + 1)]]),
                                        initial=0.0, op0=OP.mult, op1=OP.add)
                                    nc.vector.tensor_copy(
                                        out=_ap(carry, m * S + g * SG,
                                                [carry.ap[0], [1, SG], [1, 1]]),
                                        in_=_ap(h_t, TS, [h_t.ap[0], [TS + 1, SG], [1, 1]]))
                                    p_t = scpool.tile([128, SG, TS], F32, tag="p_t",
                                                      bufs=2)
                                    nc.gpsimd.tensor_tensor(
                                        out=p_t,
                                        in0=_ap(h_t, 1, [h_t.ap[0], [TS + 1, SG], [1, TS]]),
                                        in1=Cb, op=OP.mult)
                                    yg = s2pool.tile([128, TS], F32, tag="yg")
                                    nc.vector.tensor_tensor(out=yg, in0=p_t[:, 0, :],
                                                            in1=p_t[:, 1, :], op=OP.add)
                                    nc.vector.tensor_tensor(out=y_t[:, m, :],
                                                            in0=y_t[:, m, :],
                                                            in1=yg, op=OP.add)
                            # gate (z already silu'd at evac)
                            for m in range(NDI):
                                nc.vector.tensor_tensor(out=y_t[:, m, :], in0=y_t[:, m, :],
                                                        in1=z_t[:, m, :], op=OP.mult)
                            # out_proj + residual
                            for m in range(ND):
                                ops = pp.tile([128, TS], F32, tag="mm_ps")
                                for k in range(NDI):
                                    nc.tensor.matmul(ops, owt[l][k][:, m * 128:(m + 1) * 128],
                                                     y_t[:, k, :], start=(k == 0),
                                                     stop=(k == NDI - 1))
                                nc.vector.tensor_tensor(out=x_new[:, m, t0:t1],
                                                        in0=x_cur[:, m, t0:t1],
                                                        in1=ops, op=OP.add)
                            if debug and b == 0 and l == 0:
                                for m in range(NDI):
                                    dma(out=dbg["xi1"].ap().rearrange(
                                        "(n p) l -> p n l", p=128)[:, m, t0:t1],
                                        in_=xi_t[:, m, :])
                                    dma(out=dbg["dt1"].ap().rearrange(
                                        "(n p) l -> p n l", p=128)[:, m, t0:t1],
                                        in_=dt_t[:, m, :])
                                    dma(out=dbg["y1"].ap().rearrange(
                                        "(n p) l -> p n l", p=128)[:, m, t0:t1],
                                        in_=y_t[:, m, :])
                        x_cur = x_new
                        if debug and b == 0 and l == 0:
                            dma(out=dbg["x1"].ap().rearrange("(n p) l -> p n l", p=128),
                                in_=x_cur)
                    for m in range(ND):
                        mean = rpool.tile([128, 1], F32, tag="mean")
                        nc.vector.tensor_reduce(out=mean, in_=x_cur[:, m, :],
                                                axis=mybir.AxisListType.X, op=OP.add)
                        nc.scalar.activation(out=zcat[direction * ND + m][:, b:b + 1],
                                             in_=mean, func=AF.Copy, scale=1.0 / L_)

            prj = pp.tile([OUT, nb], F32, tag="mm_ps")
            for k in range(2 * ND):
                nc.tensor.matmul(prj, pwt[k], zcat[k], start=(k == 0),
                                 stop=(k == 2 * ND - 1))
            ob = rpool.tile([OUT, nb], F32, tag="out_sb")
            nc.scalar.activation(out=ob, in_=prj, func=AF.Identity, bias=pb_t[:, 0:1])
            dma(out=bass.AP(tensor=out_d.ap().tensor, offset=0,
                            ap=[[1, OUT], [OUT, nb]]), in_=ob)
    nc.compile()
    return nc


_cache = {}


def _prep_common(inputs, nlayers=NL, L_=L):
    import math
    pos = np.arange(L_, dtype=np.float32)[:, None]
    div = np.exp(np.arange(0, D, 2, dtype=np.float32) * (-math.log(10000.0) / D))
    pe = np.zeros((L_, D), np.float32)
    pe[:, 0::2] = np.sin(pos * div)
    pe[:, 1::2] = np.cos(pos * div)
    dir_emb = np.asarray(inputs["dir_emb"], np.float32)
    pe2 = np.ascontiguousarray((pe + dir_emb[0][None, :]).T)

    common = dict(
        pe2=pe2,
        ddir=np.ascontiguousarray(dir_emb[1] - dir_emb[0]),
        cont_wT=np.ascontiguousarray(np.asarray(inputs["cont_w"], np.float32).T),
        cont_b=np.asarray(inputs["cont_b"], np.float32),
        ln_g=np.asarray(inputs["ln_g"], np.float32),
        ln_b=np.asarray(inputs["ln_b"], np.float32),
        in_wT=np.ascontiguousarray(
            np.asarray(inputs["in_w"], np.float32)[:nlayers].transpose(0, 2, 1)),
        conv_w=np.ascontiguousarray(
            np.asarray(inputs["conv_w"], np.float32)[:nlayers, :, 0, :]),
        conv_b=np.asarray(inputs["conv_b"], np.float32)[:nlayers],
        xproj_wT=np.ascontiguousarray(
            np.asarray(inputs["xproj_w"], np.float32)[:nlayers].transpose(0, 2, 1)),
        dt_wT=np.ascontiguousarray(
            np.asarray(inputs["dt_w"], np.float32)[:nlayers].transpose(0, 2, 1)),
        dt_b=np.asarray(inputs["dt_b"], np.float32)[:nlayers],
        A=np.ascontiguousarray(
            -np.exp(np.asarray(inputs["A_log"], np.float32)[:nlayers])),
        Dp=np.asarray(inputs["Dp"], np.float32)[:nlayers],
        out_wT=np.ascontiguousarray(
            np.asarray(inputs["out_w"], np.float32)[:nlayers].transpose(0, 2, 1)),
        norm_g=np.asarray(inputs["norm_g"], np.float32)[:nlayers],
        norm_b=np.asarray(inputs["norm_b"], np.float32)[:nlayers],
        proj_wT=np.ascontiguousarray(np.asarray(inputs["proj_w"], np.float32).T),
        proj_b=np.asarray(inputs["proj_b"], np.float32),
    )
    sel48 = np.zeros((DTR + 2 * S, 2 * S * 128), np.float32)
    for i in range(2 * S):
        sel48[DTR + i, i * 128:(i + 1) * 128] = 1.0
    sel6 = np.zeros((6, 128), np.float32)
    sel6[5, :] = 1.0
    common["sel48"] = sel48
    common["sel6"] = sel6
    common["ones1"] = np.ones((1, 128), np.float32)
    return common


def _prep_xf(inputs, L_=L):
    x = np.asarray(inputs["x"], np.float32)
    cont_idx = [0, 1, 3, 4, 5]
    xs = x[:, :L_]
    xf = np.empty((B, 6, L_), np.float16)
    xf[:, 0:5, :] = xs[..., cont_idx].transpose(0, 2, 1)
    xf[:, 5, :] = (xs[:, :, 2] > 0).astype(np.float16)
    return xf


# weight-bearing inputs whose values feed _prep_common (everything but x)
_WKEYS = ("cont_w", "cont_b", "ln_g", "ln_b", "dir_emb", "in_w", "conv_w",
          "conv_b", "xproj_w", "dt_w", "dt_b", "A_log", "Dp", "out_w",
          "norm_g", "norm_b", "proj_w", "proj_b")


def _make_runner(nc, ncores=NCORES):
    """Persistent jit over the bass module, mirroring bass2jax.run_bass_via_pjrt
    but built once so steady-state calls skip retrace/recompile/NEFF reload."""
    import jax
    from jax.experimental.shard_map import shard_map
    from jax.sharding import Mesh, PartitionSpec, NamedSharding
    from concourse import bass2jax
    from concourse.bass2jax import _bass_exec_p, partition_id_tensor

    bass2jax.install_neuronx_cc_hook()

    partition_name = nc.partition_id_tensor.name if nc.partition_id_tensor else None
    in_names, out_names, out_avals, zero_shapes = [], [], [], []
    for alloc in nc.m.functions[0].allocations:
        if not isinstance(alloc, mybir.MemoryLocationSet):
            continue
        name = alloc.memorylocations[0].name
        if alloc.kind == "ExternalInput":
            if name != partition_name:
                in_names.append(name)
        elif alloc.kind == "ExternalOutput":
            out_names.append(name)
            shape = tuple(alloc.tensor_shape)
            dtype = mybir.dt.np(alloc.dtype)
            out_avals.append(jax.core.ShapedArray(shape, dtype))
            zero_shapes.append(((ncores * shape[0], *shape[1:]), dtype))
    n_params = len(in_names)
    n_outs = len(out_names)
    all_in = list(in_names) + list(out_names)
    if partition_name is not None:
        all_in.append(partition_name)

    def _body(*args):
        operands = list(args)
        if partition_name is not None:
            operands.append(partition_id_tensor())
        outs = _bass_exec_p.bind(
            *operands,
            out_avals=tuple(out_avals),
            in_names=tuple(all_in),
            out_names=tuple(out_names),
            lowering_input_output_aliases=(),
            sim_require_finite=True,
            sim_require_nnan=True,
            nc=nc,
        )
        return tuple(outs)

    devices = jax.devices()[:ncores]
    mesh = Mesh(np.asarray(devices), ("core",))
    P = PartitionSpec
    jitfn = jax.jit(
        shard_map(_body, mesh=mesh,
                  in_specs=(P("core"),) * (n_params + n_outs),
                  out_specs=(P("core"),) * n_outs,
                  check_rep=False),
        donate_argnums=tuple(range(n_params, n_params + n_outs)),
        keep_unused=True)
    sharding = NamedSharding(mesh, P("core"))
    return dict(jitfn=jitfn, in_names=in_names, out_names=out_names,
                zero_shapes=zero_shapes, sharding=sharding, jax=jax)


def _weights_current(inputs):
    cached = _cache.get("wraw")
    if cached is None:
        return False
    for k in _WKEYS:
        a, b = cached[k], inputs[k]
        if a is b:
            continue
        if not np.array_equal(a, np.asarray(b)):
            return False
    return True


def kernel(**inputs):
    if bool(int(os.environ.get("KERNEL_TRACE", "0"))):
        return _kernel_traced(**inputs)
    if "nc" not in _cache:
        _cache["nc"] = build()
        _cache["runner"] = _make_runner(_cache["nc"])
    run = _cache["runner"]
    jax = run["jax"]

    if not _weights_current(inputs):
        common = _prep_common(inputs)
        dev = {}
        for name in run["in_names"]:
            if name == "xf":
                continue
            w = common[name]
            rep = np.concatenate([w] * NCORES, axis=0)
            dev[name] = jax.device_put(rep, run["sharding"])
        _cache["wdev"] = dev
        _cache["wraw"] = {k: np.asarray(inputs[k]) for k in _WKEYS}

    xf = _prep_xf(inputs)
    wdev = _cache["wdev"]
    args = [xf if name == "xf" else wdev[name] for name in run["in_names"]]
    args += [np.zeros(sh, dt) for sh, dt in run["zero_shapes"]]
    outs = run["jitfn"](*args)
    oidx = run["out_names"].index("out")
    out = np.asarray(outs[oidx])          # (B, OUT), batch-concat across cores
    return np.ascontiguousarray(out.astype(np.float32))


def _kernel_traced(**inputs):
    """Profiling path: one-shot run through run_bass_kernel_spmd with trace."""
    if "nc" not in _cache:
        _cache["nc"] = build()
    nc = _cache["nc"]
    common = _prep_common(inputs)
    xf = _prep_xf(inputs)
    in_maps = []
    for c in range(NCORES):
        m = dict(common)
        m["xf"] = np.ascontiguousarray(xf[c * NB:(c + 1) * NB])
        in_maps.append(m)
    res = run_bass_kernel_spmd(nc, in_maps, core_ids=list(range(NCORES)),
                               trace=True)
    _cache["last_result"] = res
    out = np.concatenate([res.results[c]["out"] for c in range(NCORES)], axis=0)
    return np.ascontiguousarray(out.astype(np.float32))



# revision 48
# speedup vs baseline: 1.2309x; 1.0225x over previous
"""Trainium2 Bass kernel for nn_MicroBiMambaBackbone.

Sharding: pure data-parallel over batch (4 sequences per core x 8 cores).
Layout: channels on partitions, time on the free dimension.
Selective scan via DVE tensor_tensor_scan with s-major segment packing and
zero-decay boundary columns for cross-chunk state carry.
"""
import os
import sys

for _p in ("/opt/trn_rl_repo", "/root/.axon_site/_ro/trn_rl_repo"):
    if os.path.isdir(_p) and _p not in sys.path:
        sys.path.insert(0, _p)
os.environ.setdefault("MYCRO_LOCAL_CACHE", "1")

import numpy as np

import concourse.bass as bass
import concourse.bacc as bacc
import concourse.tile as tile
from concourse import mybir
from concourse.bass_utils import run_bass_kernel_spmd

F32 = mybir.dt.float32
F16 = mybir.dt.float16
AF = mybir.ActivationFunctionType
OP = mybir.AluOpType

# model dims
B, L, DIN = 32, 1024, 6
D, DI, S, K, DTR = 256, 512, 16, 4, 16
NL = 4
OUT = 128
NCORES = 8
NB = B // NCORES          # sequences per core
ND = D // 128             # d-tiles of model dim
NDI = DI // 128           # d-tiles of inner dim
TS = 512                  # time slab
NSLAB = L // TS
SG = 2                    # s-group size for scan ops
NSG = S // SG
EPS = 1e-5


def _ap(t, offset_delta, dims):
    return bass.AP(tensor=t.tensor, offset=t.offset + offset_delta, ap=dims)


# engine-assignment tuning knobs (sim-swept): 1 = alternate DVE/Pool by m
# parity, 0 = all DVE
TUNE = dict(scan_alt=0, bp_alt=1, ln_alt=0, conv_alt=0, dt_alt=0, gate_alt=0,
            out_alt=0, aux_pool=1)


def build(nb=NB, nlayers=NL, nslab=NSLAB, debug=False):
    nc = bacc.Bacc("TRN2", target_bir_lowering=False, debug=False)
    L_ = nslab * TS

    xf_d = nc.dram_tensor("xf", [nb, 6, L_], F16, kind="ExternalInput")
    pe2_d = nc.dram_tensor("pe2", [D, L_], F32, kind="ExternalInput")
    ddir_d = nc.dram_tensor("ddir", [D], F32, kind="ExternalInput")
    cwt_d = nc.dram_tensor("cont_wT", [5, D], F32, kind="ExternalInput")
    cb_d = nc.dram_tensor("cont_b", [D], F32, kind="ExternalInput")
    lng_d = nc.dram_tensor("ln_g", [D], F32, kind="ExternalInput")
    lnb_d = nc.dram_tensor("ln_b", [D], F32, kind="ExternalInput")
    inwt_d = nc.dram_tensor("in_wT", [nlayers, D, 2 * DI], F16, kind="ExternalInput")
    cvw_d = nc.dram_tensor("conv_w", [nlayers, DI, K], F32, kind="ExternalInput")
    cvb_d = nc.dram_tensor("conv_b", [nlayers, DI], F32, kind="ExternalInput")
    xpt_d = nc.dram_tensor("xproj_wT", [nlayers, DI, DTR + 2 * S], F16, kind="ExternalInput")
    dtwt_d = nc.dram_tensor("dt_wT", [nlayers, DTR, DI], F16, kind="ExternalInput")
    dtb_d = nc.dram_tensor("dt_b", [nlayers, DI], F32, kind="ExternalInput")
    A_d = nc.dram_tensor("A", [nlayers, DI, S], F32, kind="ExternalInput")
    Dp_d = nc.dram_tensor("Dp", [nlayers, DI], F32, kind="ExternalInput")
    owt_d = nc.dram_tensor("out_wT", [nlayers, DI, D], F16, kind="ExternalInput")
    ng_d = nc.dram_tensor("norm_g", [nlayers, D], F32, kind="ExternalInput")
    nb_d = nc.dram_tensor("norm_b", [nlayers, D], F32, kind="ExternalInput")
    pwt_d = nc.dram_tensor("proj_wT", [2 * D, OUT], F32, kind="ExternalInput")
    pb_d = nc.dram_tensor("proj_b", [OUT], F32, kind="ExternalInput")
    ones1_d = nc.dram_tensor("ones1", [1, 128], F32, kind="ExternalInput")
    selc_d = nc.dram_tensor("selc", [DTR + 2 * S, 2 * S], F16, kind="ExternalInput")
    sel6c_d = nc.dram_tensor("sel6c", [6, 1], F32, kind="ExternalInput")

    out_d = nc.dram_tensor("out", [nb, OUT], F32, kind="ExternalOutput")
    dbg = {}
    if debug:
        for nm, sh in (("h0", [D, L_]), ("x1", [D, L_]), ("xi1", [DI, L_]),
                       ("dt1", [DI, L_]), ("y1", [DI, L_])):
            dbg[nm] = nc.dram_tensor("dbg_" + nm, sh, F32, kind="ExternalOutput")

    with tile.TileContext(nc) as tc:
        import contextlib
        with contextlib.ExitStack() as ctx:
            wpool = ctx.enter_context(tc.tile_pool(name="weights", bufs=1))
            wstr = ctx.enter_context(tc.tile_pool(name="wstream", bufs=1))
            apool = ctx.enter_context(tc.tile_pool(name="acts", bufs=1))
            spool = ctx.enter_context(tc.tile_pool(name="slab", bufs=1))
            s2pool = ctx.enter_context(tc.tile_pool(name="slab2", bufs=1))
            scpool = ctx.enter_context(tc.tile_pool(name="scan", bufs=1))
            rpool = ctx.enter_context(tc.tile_pool(name="rows", bufs=1))
            pp = ctx.enter_context(tc.tile_pool(name="ps_mm", bufs=1, space="PSUM"))
            pln = ctx.enter_context(tc.tile_pool(name="ps_ln", bufs=1, space="PSUM"))
            pbc = ctx.enter_context(tc.tile_pool(name="ps_bc", bufs=1, space="PSUM"))

            dma = nc.gpsimd.dma_start

            _wn = [0]

            def loadw(dram_ap, shape, dt=F32):
                _wn[0] += 1
                t = wpool.tile(shape, dt, name=f"w{_wn[0]}", tag=f"w{_wn[0]}")
                dma(out=t, in_=dram_ap)
                return t

            ones1 = loadw(ones1_d.ap(), [1, 128])
            selc = loadw(selc_d.ap(), [DTR + 2 * S, 2 * S], F16)
            sel6c = loadw(sel6c_d.ap(), [6, 1])

            def bc_stat(col):
                return bass.AP(tensor=selc.tensor, offset=selc.offset + col,
                               ap=[[2 * S, DTR + 2 * S], [0, 128]])
            onescol = wpool.tile([128, 1], F32)
            nc.vector.memset(onescol, 1.0)
            eps_t = wpool.tile([1, 1], F32)
            nc.vector.memset(eps_t, EPS)

            cwt = [loadw(cwt_d.ap()[:, m * 128:(m + 1) * 128], [5, 128]) for m in range(ND)]
            pe2 = loadw(pe2_d.ap().rearrange("(n p) l -> p n l", p=128), [128, ND, L_])

            def load_cols(dram_t, n, base):
                _wn[0] += 1
                t = wpool.tile([128, n], F32, name=f"w{_wn[0]}", tag=f"w{_wn[0]}")
                dma(out=t, in_=bass.AP(tensor=dram_t.ap().tensor, offset=base,
                                       ap=[[1, 128], [128, n]]))
                return t

            cont_b = load_cols(cb_d, ND, 0)
            ln_g = load_cols(lng_d, ND, 0)
            ln_b = load_cols(lnb_d, ND, 0)
            ddir = load_cols(ddir_d, ND, 0)
            pb_t = load_cols(pb_d, 1, 0)

            xpt = [[loadw(xpt_d.ap()[l, k * 128:(k + 1) * 128, :], [128, DTR + 2 * S], F16)
                    for k in range(NDI)] for l in range(nlayers)]
            dtwt = [loadw(dtwt_d.ap()[l], [DTR, DI], F16) for l in range(nlayers)]
            owt = [[loadw(owt_d.ap()[l, k * 128:(k + 1) * 128, :], [128, D], F16)
                    for k in range(NDI)] for l in range(nlayers)]
            pwt = [loadw(pwt_d.ap()[k * 128:(k + 1) * 128, :], [128, OUT])
                   for k in range(2 * ND)]

            def load_convw(l, m):
                _wn[0] += 1
                t = wpool.tile([128, K], F32, name=f"w{_wn[0]}", tag=f"w{_wn[0]}")
                dma(out=t, in_=bass.AP(tensor=cvw_d.ap().tensor,
                                       offset=(l * DI + m * 128) * K,
                                       ap=[[K, 128], [1, K]]))
                return t

            cvw = [[load_convw(l, m) for m in range(NDI)] for l in range(nlayers)]
            cvb = [load_cols(cvb_d, NDI, l * DI) for l in range(nlayers)]
            dtb = [load_cols(dtb_d, NDI, l * DI) for l in range(nlayers)]
            Dpw = [load_cols(Dp_d, NDI, l * DI) for l in range(nlayers)]
            ng = [load_cols(ng_d, ND, l * D) for l in range(nlayers)]
            nbt = [load_cols(nb_d, ND, l * D) for l in range(nlayers)]
            A_t = [[loadw(A_d.ap()[l, m * 128:(m + 1) * 128, :], [128, S])
                    for m in range(NDI)] for l in range(nlayers)]

            zcat = [apool.tile([128, nb], F32, tag=f"zcat{k}", name=f"zcat{k}") for k in range(2 * ND)]

            def layer_norm(x_aps, g_cols, b_cols, out_aps):
                ssum = pln.tile([1, TS], F32, tag="ln_sum")
                s2 = pln.tile([1, TS], F32, tag="ln_sum2")
                sqt = rpool.tile([128, TS], F32, tag="ln_sq")
                for i, xt in enumerate(x_aps):
                    nc.scalar.activation(out=sqt, in_=xt, func=AF.Square)
                    nc.tensor.matmul(s2, onescol, sqt,
                                     start=(i == 0), stop=(i == len(x_aps) - 1))
                for i, xt in enumerate(x_aps):
                    nc.tensor.matmul(ssum, onescol, xt,
                                     start=(i == 0), stop=(i == len(x_aps) - 1))
                murs = rpool.tile([1, 2 * TS], F32, tag="ln_murs")
                nc.scalar.activation(out=murs[:, 0:TS], in_=ssum, func=AF.Copy,
                                     scale=1.0 / D)
                r1 = rpool.tile([1, TS], F32, tag="ln_r1")
                nc.scalar.activation(out=r1, in_=s2, func=AF.Copy, scale=1.0 / D)
                r2 = rpool.tile([1, TS], F32, tag="ln_r2")
                nc.scalar.activation(out=r2, in_=murs[:, 0:TS], func=AF.Square)
                nc.vector.tensor_tensor(out=r1, in0=r1, in1=r2, op=OP.subtract)
                nc.scalar.activation(out=r1, in_=r1, func=AF.Ln, bias=eps_t[0:1, 0:1])
                nc.scalar.activation(out=murs[:, TS:], in_=r1, func=AF.Exp, scale=-0.5)
                lnbc = pln.tile([128, TS], F32, tag="ln_bc")
                nc.tensor.matmul(lnbc, ones1, murs[:, 0:TS], start=True, stop=True)
                for i, xt in enumerate(x_aps):
                    eng = nc.gpsimd if (TUNE["ln_alt"] and i % 2) else nc.vector
                    eng.tensor_tensor(out=out_aps[i], in0=xt, in1=lnbc,
                                      op=OP.subtract)
                lnbc2 = pln.tile([128, TS], F32, tag="ln_bc")
                nc.tensor.matmul(lnbc2, ones1, murs[:, TS:], start=True, stop=True)
                for i in range(len(x_aps)):
                    eng = nc.gpsimd if (TUNE["ln_alt"] and i % 2) else nc.vector
                    eng.tensor_tensor(out=out_aps[i], in0=out_aps[i], in1=lnbc2,
                                      op=OP.mult)
                    eng.tensor_scalar(out=out_aps[i], in0=out_aps[i],
                                      scalar1=g_cols[:, i:i + 1],
                                      scalar2=b_cols[:, i:i + 1],
                                      op0=OP.mult, op1=OP.add)

            for b in range(nb):
                # ===== embedding =====
                xf16 = apool.tile([6, L_], F16, tag="xf16", bufs=2)
                dma(out=xf16, in_=xf_d.ap()[b])
                xf = apool.tile([6, L_], F32, tag="xf")
                nc.vector.tensor_copy(out=xf, in_=xf16)
                h_fwd = apool.tile([128, ND, L_], F32, tag="h_fwd", bufs=2)
                h_rev = apool.tile([128, ND, L_], F32, tag="h_rev", bufs=2)
                for islab in range(nslab):
                    t0, t1 = islab * TS, (islab + 1) * TS
                    e_sb = spool.tile([128, ND, TS], F32, tag="emb_e")
                    for m in range(ND):
                        ep = pp.tile([128, TS], F32, tag="mm_ps")
                        nc.tensor.matmul(ep, cwt[m], xf[0:5, t0:t1], start=True, stop=True)
                        nc.scalar.activation(out=e_sb[:, m, :], in_=ep, func=AF.Identity,
                                             bias=cont_b[:, m:m + 1])
                    xn = spool.tile([128, ND, TS], F32, tag="xn_e")
                    layer_norm([e_sb[:, m, :] for m in range(ND)], ln_g, ln_b,
                               [xn[:, m, :] for m in range(ND)])
                    mb = pp.tile([128, TS], F32, tag="mm_ps")
                    nc.tensor.matmul(
                        mb,
                        bass.AP(tensor=sel6c.tensor, offset=sel6c.offset,
                                ap=[[1, 6], [0, 128]]),
                        xf[:, t0:t1], start=True, stop=True)
                    for m in range(ND):
                        nc.scalar.activation(out=xn[:, m, :], in_=xn[:, m, :],
                                             func=AF.Gelu)
                        hm = h_fwd[:, m, t0:t1]
                        nc.vector.tensor_tensor(out=hm, in0=xn[:, m, :],
                                                in1=pe2[:, m, t0:t1], op=OP.add)
                        nc.vector.scalar_tensor_tensor(out=hm, in0=mb,
                                                       scalar=ddir[:, m:m + 1],
                                                       in1=hm, op0=OP.mult, op1=OP.add)
                for m in range(ND):
                    src = _ap(h_fwd, m * L_ + (L_ - 1), [h_fwd.ap[0], [-1, L_]])
                    nc.vector.tensor_copy(out=h_rev[:, m, :], in_=src)
                if debug and b == 0:
                    dma(out=dbg["h0"].ap().rearrange("(n p) l -> p n l", p=128), in_=h_fwd)

                # ===== mamba stacks =====
                for direction in range(2):
                    x_cur = h_fwd if direction == 0 else h_rev
                    lrange = (range(0, nlayers - nlayers // 2) if direction == 0
                              else range(nlayers - nlayers // 2, nlayers))
                    for li, l in enumerate(lrange):
                        inw = wstr.tile([128, ND, 2 * DI], F16, tag="inw")
                        dma(out=inw, in_=inwt_d.ap()[l].rearrange(
                            "(n p) e -> p n e", p=128))
                        if li == 0:
                            x_new = apool.tile([128, ND, L_], F32, tag="xnew0",
                                               bufs=2)
                        else:
                            x_new = h_fwd if direction == 0 else h_rev
                        carry = apool.tile([128, NDI, S], F16, tag="carry")
                        nc.vector.memset(carry, 0.0)
                        halo = apool.tile([128, NDI, K - 1], F16, tag="halo")
                        nc.vector.memset(halo, 0.0)
                        for islab in range(nslab):
                            t0, t1 = islab * TS, (islab + 1) * TS
                            xn = spool.tile([128, ND, TS], F16, tag="xn")
                            layer_norm([x_cur[:, m, t0:t1] for m in range(ND)],
                                       ng[l], nbt[l],
                                       [xn[:, m, :] for m in range(ND)])
                            xi_raw = spool.tile([128, NDI, K - 1 + TS], F16, tag="xi_raw")
                            z_t = spool.tile([128, NDI, TS], F16, tag="z")
                            xi_t = spool.tile([128, NDI, TS], F16, tag="xi")
                            dt_t = spool.tile([128, NDI, TS], F16, tag="dt")
                            y_t = spool.tile([128, NDI, TS], F16, tag="y")
                            nc.vector.tensor_copy(
                                out=_ap(xi_raw, 0,
                                        [xi_raw.ap[0], [K - 1 + TS, NDI], [1, K - 1]]),
                                in_=halo)
                            for m in range(2 * NDI):
                                psm = pp.tile([128, TS], F32, tag="mm_ps")
                                for k in range(ND):
                                    nc.tensor.matmul(psm, inw[:, k, m * 128:(m + 1) * 128],
                                                     xn[:, k, :], start=(k == 0),
                                                     stop=(k == ND - 1))
                                if m < NDI:
                                    nc.scalar.activation(out=xi_raw[:, m, K - 1:],
                                                         in_=psm, func=AF.Copy)
                                else:
                                    # silu fused into the evacuation
                                    nc.scalar.activation(out=z_t[:, m - NDI, :],
                                                         in_=psm, func=AF.Silu)
                            nc.vector.tensor_copy(
                                out=halo,
                                in_=_ap(xi_raw, TS,
                                        [xi_raw.ap[0], [K - 1 + TS, NDI], [1, K - 1]]))
                            # conv + silu (z already silu'd at evac)
                            for m in range(NDI):
                                ceng = (nc.gpsimd if (TUNE["conv_alt"] and m % 2)
                                        else nc.vector)
                                acc = s2pool.tile([128, TS], F16, tag="convacc")
                                ceng.tensor_scalar(out=acc, in0=xi_raw[:, m, K - 1:],
                                                   scalar1=cvw[l][m][:, K - 1:K],
                                                   scalar2=None, op0=OP.mult)
                                for kk in range(K - 2, -1, -1):
                                    ceng.scalar_tensor_tensor(
                                        out=acc, in0=xi_raw[:, m, kk:kk + TS],
                                        scalar=cvw[l][m][:, kk:kk + 1],
                                        in1=acc, op0=OP.mult, op1=OP.add)
                                nc.scalar.activation(out=xi_t[:, m, :], in_=acc,
                                                     func=AF.Silu, bias=cvb[l][:, m:m + 1])
                            # xproj
                            xdb_ps = pp.tile([DTR + 2 * S, TS], F32, tag="mm_ps")
                            for k in range(NDI):
                                nc.tensor.matmul(xdb_ps, xpt[l][k], xi_t[:, k, :],
                                                 start=(k == 0), stop=(k == NDI - 1))
                            xdb = s2pool.tile([DTR + 2 * S, TS], F16, tag="xdb")
                            nc.scalar.activation(out=xdb, in_=xdb_ps, func=AF.Copy)
                            # dt proj + softplus; dtu
                            for m in range(NDI):
                                deng = (nc.gpsimd if (TUNE["dt_alt"] and m % 2)
                                        else nc.vector)
                                dps = pp.tile([128, TS], F32, tag="mm_ps")
                                nc.tensor.matmul(dps, dtwt[l][:, m * 128:(m + 1) * 128],
                                                 xdb[0:DTR, :], start=True, stop=True)
                                spx = s2pool.tile([128, TS], F32, tag="spx")
                                nc.scalar.activation(out=spx, in_=dps, func=AF.Exp,
                                                     bias=dtb[l][:, m:m + 1])
                                nc.scalar.activation(out=dt_t[:, m, :], in_=spx,
                                                     func=AF.Ln, bias=onescol[:, 0:1])
                                deng.tensor_scalar(out=y_t[:, m, :],
                                                   in0=xi_t[:, m, :],
                                                   scalar1=Dpw[l][:, m:m + 1],
                                                   scalar2=None, op0=OP.mult)
                                deng.tensor_tensor(out=xi_t[:, m, :],
                                                   in0=xi_t[:, m, :],
                                                   in1=dt_t[:, m, :], op=OP.mult)
                            # scan over s-groups: B/C matmuls into one 4-bank
                            # PSUM tile, single Act evac to f16 SBUF per group
                            for g in range(NSG):
                                BCps = pbc.tile([128, 2 * SG, TS], F32, tag="BCps")
                                for j in range(SG):
                                    s = g * SG + j
                                    nc.tensor.matmul(BCps[:, j, :], bc_stat(s),
                                                     xdb, start=True, stop=True)
                                    nc.tensor.matmul(BCps[:, SG + j, :],
                                                     bc_stat(S + s),
                                                     xdb, start=True, stop=True)
                                BCb = scpool.tile([128, 2 * SG, TS], F16, tag="BCb",
                                                  bufs=2)
                                nc.scalar.activation(out=BCb, in_=BCps, func=AF.Copy)
                                for m in range(NDI):
                                    seng = nc.vector
                                    beng = (nc.gpsimd if (TUNE["bp_alt"] and m % 2)
                                            else nc.vector)
                                    peng = (nc.gpsimd if (TUNE["bp_alt"] and m % 2 == 0)
                                            else nc.vector)
                                    aux = nc.gpsimd if TUNE["aux_pool"] else nc.vector
                                    a_t = scpool.tile([128, SG, TS + 1], F16, tag="a_t", bufs=2)
                                    b_t = scpool.tile([128, SG, TS + 1], F16, tag="b_t", bufs=2)
                                    h_t = scpool.tile([128, SG, TS + 1], F16, tag="h_t", bufs=2)
                                    for j in range(SG):
                                        s = g * SG + j
                                        nc.scalar.activation(out=a_t[:, j, 1:],
                                                             in_=dt_t[:, m, :],
                                                             func=AF.Exp,
                                                             scale=A_t[l][m][:, s:s + 1])
                                    aux.memset(
                                        _ap(a_t, 0, [a_t.ap[0], [TS + 1, SG], [1, 1]]), 0.0)
                                    aux.tensor_copy(
                                        out=_ap(b_t, 0, [b_t.ap[0], [TS + 1, SG], [1, 1]]),
                                        in_=_ap(carry, m * S + g * SG,
                                                [carry.ap[0], [1, SG], [1, 1]]))
                                    dtu_rep = _ap(xi_t, m * TS,
                                                  [xi_t.ap[0], [0, SG], [1, TS]])
                                    beng.tensor_tensor(
                                        out=_ap(b_t, 1, [b_t.ap[0], [TS + 1, SG], [1, TS]]),
                                        in0=dtu_rep, in1=BCb[:, 0:SG, :], op=OP.mult)
                                    seng.tensor_tensor_scan(
                                        out=_ap(h_t, 0, [h_t.ap[0], [1, SG * (TS + 1)]]),
                                        data0=_ap(a_t, 0, [a_t.ap[0], [1, SG * (TS + 1)]]),
                                        data1=_ap(b_t, 0, [b_t.ap[0], [1, SG * (TS + 1)]]),
                                        initial=0.0, op0=OP.mult, op1=OP.add)
                                    aux.tensor_copy(
                                        out=_ap(carry, m * S + g * SG,
                                                [carry.ap[0], [1, SG], [1, 1]]),
                                        in_=_ap(h_t, TS, [h_t.ap[0], [TS + 1, SG], [1, 1]]))
                                    p_t = scpool.tile([128, SG, TS], F16, tag="p_t")
                                    peng.tensor_tensor(
                                        out=p_t,
                                        in0=_ap(h_t, 1, [h_t.ap[0], [TS + 1, SG], [1, TS]]),
                                        in1=BCb[:, SG:2 * SG, :], op=OP.mult)
                                    yg = s2pool.tile([128, TS], F16, tag="yg")
                                    nc.vector.tensor_tensor(out=yg, in0=p_t[:, 0, :],
                                                            in1=p_t[:, 1, :], op=OP.add)
                                    nc.vector.tensor_tensor(out=y_t[:, m, :],
                                                            in0=y_t[:, m, :],
                                                            in1=yg, op=OP.add)
                            # gate (z already silu'd at evac)
                            for m in range(NDI):
                                geng = (nc.gpsimd if (TUNE["gate_alt"] and m % 2)
                                        else nc.vector)
                                geng.tensor_tensor(out=y_t[:, m, :], in0=y_t[:, m, :],
                                                   in1=z_t[:, m, :], op=OP.mult)
                            # out_proj + residual
                            for m in range(ND):
                                ops = pp.tile([128, TS], F32, tag="mm_ps")
                                for k in range(NDI):
                                    nc.tensor.matmul(ops, owt[l][k][:, m * 128:(m + 1) * 128],
                                                     y_t[:, k, :], start=(k == 0),
                                                     stop=(k == NDI - 1))
                                oeng = (nc.gpsimd if (TUNE["out_alt"] and m % 2)
                                        else nc.vector)
                                oeng.tensor_tensor(out=x_new[:, m, t0:t1],
                                                   in0=x_cur[:, m, t0:t1],
                                                   in1=ops, op=OP.add)
                            if debug and b == 0 and l == 0:
                                for m in range(NDI):
                                    dma(out=dbg["xi1"].ap().rearrange(
                                        "(n p) l -> p n l", p=128)[:, m, t0:t1],
                                        in_=xi_t[:, m, :])
                                    dma(out=dbg["dt1"].ap().rearrange(
                                        "(n p) l -> p n l", p=128)[:, m, t0:t1],
                                        in_=dt_t[:, m, :])
                                    dma(out=dbg["y1"].ap().rearrange(
                                        "(n p) l -> p n l", p=128)[:, m, t0:t1],
                                        in_=y_t[:, m, :])
                        x_cur = x_new
                        if debug and b == 0 and l == 0:
                            dma(out=dbg["x1"].ap().rearrange("(n p) l -> p n l", p=128),
                                in_=x_cur)
                    for m in range(ND):
                        mean = rpool.tile([128, 1], F32, tag="mean")
                        nc.vector.tensor_reduce(out=mean, in_=x_cur[:, m, :],
                                                axis=mybir.AxisListType.X, op=OP.add)
                        nc.scalar.activation(out=zcat[direction * ND + m][:, b:b + 1],
                                             in_=mean, func=AF.Copy, scale=1.0 / L_)

            prj = pp.tile([OUT, nb], F32, tag="mm_ps")
            for k in range(2 * ND):
                nc.tensor.matmul(prj, pwt[k], zcat[k], start=(k == 0),
                                 stop=(k == 2 * ND - 1))
            ob = rpool.tile([OUT, nb], F32, tag="out_sb")
            nc.scalar.activation(out=ob, in_=prj, func=AF.Identity, bias=pb_t[:, 0:1])
            dma(out=bass.AP(tensor=out_d.ap().tensor, offset=0,
                            ap=[[1, OUT], [OUT, nb]]), in_=ob)
    nc.compile()
    return nc


_cache = {}


def _prep_common(inputs, nlayers=NL, L_=L):
    import math
    pos = np.arange(L_, dtype=np.float32)[:, None]
    div = np.exp(np.arange(0, D, 2, dtype=np.float32) * (-math.log(10000.0) / D))
    pe = np.zeros((L_, D), np.float32)
    pe[:, 0::2] = np.sin(pos * div)
    pe[:, 1::2] = np.cos(pos * div)
    dir_emb = np.asarray(inputs["dir_emb"], np.float32)
    pe2 = np.ascontiguousarray((pe + dir_emb[0][None, :]).T)

    common = dict(
        pe2=pe2,
        ddir=np.ascontiguousarray(dir_emb[1] - dir_emb[0]),
        cont_wT=np.ascontiguousarray(np.asarray(inputs["cont_w"], np.float32).T),
        cont_b=np.asarray(inputs["cont_b"], np.float32),
        ln_g=np.asarray(inputs["ln_g"], np.float32),
        ln_b=np.asarray(inputs["ln_b"], np.float32),
        in_wT=np.ascontiguousarray(
            np.asarray(inputs["in_w"], np.float32)[:nlayers].transpose(0, 2, 1)).astype(np.float16),
        conv_w=np.ascontiguousarray(
            np.asarray(inputs["conv_w"], np.float32)[:nlayers, :, 0, :]),
        conv_b=np.asarray(inputs["conv_b"], np.float32)[:nlayers],
        xproj_wT=np.ascontiguousarray(
            np.asarray(inputs["xproj_w"], np.float32)[:nlayers].transpose(0, 2, 1)).astype(np.float16),
        dt_wT=np.ascontiguousarray(
            np.asarray(inputs["dt_w"], np.float32)[:nlayers].transpose(0, 2, 1)).astype(np.float16),
        dt_b=np.asarray(inputs["dt_b"], np.float32)[:nlayers],
        A=np.ascontiguousarray(
            -np.exp(np.asarray(inputs["A_log"], np.float32)[:nlayers])),
        Dp=np.asarray(inputs["Dp"], np.float32)[:nlayers],
        out_wT=np.ascontiguousarray(
            np.asarray(inputs["out_w"], np.float32)[:nlayers].transpose(0, 2, 1)).astype(np.float16),
        norm_g=np.asarray(inputs["norm_g"], np.float32)[:nlayers],
        norm_b=np.asarray(inputs["norm_b"], np.float32)[:nlayers],
        proj_wT=np.ascontiguousarray(np.asarray(inputs["proj_w"], np.float32).T),
        proj_b=np.asarray(inputs["proj_b"], np.float32),
    )
    common["ones1"] = np.ones((1, 128), np.float32)
    selc = np.zeros((DTR + 2 * S, 2 * S), np.float16)
    for i in range(2 * S):
        selc[DTR + i, i] = 1.0
    sel6c = np.zeros((6, 1), np.float32)
    sel6c[5, 0] = 1.0
    common["selc"] = selc
    common["sel6c"] = sel6c
    return common


def _prep_xf(inputs, L_=L):
    x = np.asarray(inputs["x"], np.float32)
    cont_idx = [0, 1, 3, 4, 5]
    xs = x[:, :L_]
    xf = np.empty((B, 6, L_), np.float16)
    xf[:, 0:5, :] = xs[..., cont_idx].transpose(0, 2, 1)
    xf[:, 5, :] = (xs[:, :, 2] > 0).astype(np.float16)
    return xf


# weight-bearing inputs whose values feed _prep_common (everything but x)
_WKEYS = ("cont_w", "cont_b", "ln_g", "ln_b", "dir_emb", "in_w", "conv_w",
          "conv_b", "xproj_w", "dt_w", "dt_b", "A_log", "Dp", "out_w",
          "norm_g", "norm_b", "proj_w", "proj_b")


def _make_runner(nc, ncores=NCORES):
    """Persistent jit over the bass module, mirroring bass2jax.run_bass_via_pjrt
    but built once so steady-state calls skip retrace/recompile/NEFF reload."""
    import jax
    from jax.experimental.shard_map import shard_map
    from jax.sharding import Mesh, PartitionSpec, NamedSharding
    from concourse import bass2jax
    from concourse.bass2jax import _bass_exec_p, partition_id_tensor

    bass2jax.install_neuronx_cc_hook()

    partition_name = nc.partition_id_tensor.name if nc.partition_id_tensor else None
    in_names, out_names, out_avals, zero_shapes = [], [], [], []
    for alloc in nc.m.functions[0].allocations:
        if not isinstance(alloc, mybir.MemoryLocationSet):
            continue
        name = alloc.memorylocations[0].name
        if alloc.kind == "ExternalInput":
            if name != partition_name:
                in_names.append(name)
        elif alloc.kind == "ExternalOutput":
            out_names.append(name)
            shape = tuple(alloc.tensor_shape)
            dtype = mybir.dt.np(alloc.dtype)
            out_avals.append(jax.core.ShapedArray(shape, dtype))
            zero_shapes.append(((ncores * shape[0], *shape[1:]), dtype))
    n_params = len(in_names)
    n_outs = len(out_names)
    all_in = list(in_names) + list(out_names)
    if partition_name is not None:
        all_in.append(partition_name)

    def _body(*args):
        operands = list(args)
        if partition_name is not None:
            operands.append(partition_id_tensor())
        outs = _bass_exec_p.bind(
            *operands,
            out_avals=tuple(out_avals),
            in_names=tuple(all_in),
            out_names=tuple(out_names),
            lowering_input_output_aliases=(),
            sim_require_finite=True,
            sim_require_nnan=True,
            nc=nc,
        )
        return tuple(outs)

    devices = jax.devices()[:ncores]
    mesh = Mesh(np.asarray(devices), ("core",))
    P = PartitionSpec
    jitfn = jax.jit(
        shard_map(_body, mesh=mesh,
                  in_specs=(P("core"),) * (n_params + n_outs),
                  out_specs=(P("core"),) * n_outs,
                  check_rep=False),
        donate_argnums=tuple(range(n_params, n_params + n_outs)),
        keep_unused=True)
    sharding = NamedSharding(mesh, P("core"))
    return dict(jitfn=jitfn, in_names=in_names, out_names=out_names,
                zero_shapes=zero_shapes, sharding=sharding, jax=jax)


def _weights_current(inputs):
    cached = _cache.get("wraw")
    if cached is None:
        return False
    for k in _WKEYS:
        a, b = cached[k], inputs[k]
        if a is b:
            continue
        if not np.array_equal(a, np.asarray(b)):
            return False
    return True


def kernel(**inputs):
    if bool(int(os.environ.get("KERNEL_TRACE", "0"))):
        return _kernel_traced(**inputs)
    if "nc" not in _cache:
        _cache["nc"] = build()
        _cache["runner"] = _make_runner(_cache["nc"])
    run = _cache["runner"]
    jax = run["jax"]

    if not _weights_current(inputs):
        common = _prep_common(inputs)
        dev = {}
        for name in run["in_names"]:
            if name == "xf":
                continue
            w = common[name]
            rep = np.concatenate([w] * NCORES, axis=0)
            dev[name] = jax.device_put(rep, run["sharding"])
        _cache["wdev"] = dev
        _cache["wraw"] = {k: np.asarray(inputs[k]) for k in _WKEYS}

    xf = _prep_xf(inputs)
    wdev = _cache["wdev"]
    args = [xf if name == "xf" else wdev[name] for name in run["in_names"]]
    args += [np.zeros(sh, dt) for sh, dt in run["zero_shapes"]]
    outs = run["jitfn"](*args)
    oidx = run["out_names"].index("out")
    out = np.asarray(outs[oidx])          # (B, OUT), batch-concat across cores
    return np.ascontiguousarray(out.astype(np.float32))


def _kernel_traced(**inputs):
    """Profiling path: one-shot run through run_bass_kernel_spmd with trace."""
    if "nc" not in _cache:
        _cache["nc"] = build()
    nc = _cache["nc"]
    common = _prep_common(inputs)
    xf = _prep_xf(inputs)
    in_maps = []
    for c in range(NCORES):
        m = dict(common)
        m["xf"] = np.ascontiguousarray(xf[c * NB:(c + 1) * NB])
        in_maps.append(m)
    res = run_bass_kernel_spmd(nc, in_maps, core_ids=list(range(NCORES)),
                               trace=True)
    _cache["last_result"] = res
    out = np.concatenate([res.results[c]["out"] for c in range(NCORES)], axis=0)
    return np.ascontiguousarray(out.astype(np.float32))



# revision 53
# speedup vs baseline: 1.2883x; 1.0466x over previous
"""Trainium2 Bass kernel for nn_MicroBiMambaBackbone.

Sharding: pure data-parallel over batch (4 sequences per core x 8 cores).
Layout: channels on partitions, time on the free dimension.
Selective scan via DVE tensor_tensor_scan with s-major segment packing and
zero-decay boundary columns for cross-chunk state carry.
"""
import os
import sys

for _p in ("/opt/trn_rl_repo", "/root/.axon_site/_ro/trn_rl_repo"):
    if os.path.isdir(_p) and _p not in sys.path:
        sys.path.insert(0, _p)
os.environ.setdefault("MYCRO_LOCAL_CACHE", "1")

import numpy as np

import concourse.bass as bass
import concourse.bacc as bacc
import concourse.tile as tile
from concourse import mybir
from concourse.bass_utils import run_bass_kernel_spmd

F32 = mybir.dt.float32
F16 = mybir.dt.float16
AF = mybir.ActivationFunctionType
OP = mybir.AluOpType

# model dims
B, L, DIN = 32, 1024, 6
D, DI, S, K, DTR = 256, 512, 16, 4, 16
NL = 4
OUT = 128
NCORES = 8
NB = B // NCORES          # sequences per core
ND = D // 128             # d-tiles of model dim
NDI = DI // 128           # d-tiles of inner dim
TS = 512                  # time slab
NSLAB = L // TS
SG = 2                    # s-group size for scan ops
NSG = S // SG
EPS = 1e-5


def _ap(t, offset_delta, dims):
    return bass.AP(tensor=t.tensor, offset=t.offset + offset_delta, ap=dims)


# engine-assignment tuning knobs (sim-swept): 1 = alternate DVE/Pool by m
# parity, 0 = all DVE
TUNE = dict(scan_alt=0, bp_alt=1, ln_alt=0, conv_alt=0, dt_alt=0, gate_alt=0,
            out_alt=0, aux_pool=1)


def build(nb=NB, nlayers=NL, nslab=NSLAB, debug=False):
    nc = bacc.Bacc("TRN2", target_bir_lowering=False, debug=False)
    L_ = nslab * TS

    xf_d = nc.dram_tensor("xf", [nb, 6, L_], F16, kind="ExternalInput")
    pe2_d = nc.dram_tensor("pe2", [D, L_], F32, kind="ExternalInput")
    ddir_d = nc.dram_tensor("ddir", [D], F32, kind="ExternalInput")
    cwt_d = nc.dram_tensor("cont_wT", [5, D], F32, kind="ExternalInput")
    cb_d = nc.dram_tensor("cont_b", [D], F32, kind="ExternalInput")
    lng_d = nc.dram_tensor("ln_g", [D], F32, kind="ExternalInput")
    lnb_d = nc.dram_tensor("ln_b", [D], F32, kind="ExternalInput")
    inwt_d = nc.dram_tensor("in_wT", [nlayers, D, 2 * DI], F16, kind="ExternalInput")
    cvw_d = nc.dram_tensor("conv_w", [nlayers, DI, K], F32, kind="ExternalInput")
    cvb_d = nc.dram_tensor("conv_b", [nlayers, DI], F32, kind="ExternalInput")
    xpt_d = nc.dram_tensor("xproj_wT", [nlayers, DI, DTR + 2 * S], F16, kind="ExternalInput")
    dtwt_d = nc.dram_tensor("dt_wT", [nlayers, DTR, DI], F16, kind="ExternalInput")
    dtb_d = nc.dram_tensor("dt_b", [nlayers, DI], F32, kind="ExternalInput")
    A_d = nc.dram_tensor("A", [nlayers, DI, S], F32, kind="ExternalInput")
    Dp_d = nc.dram_tensor("Dp", [nlayers, DI], F32, kind="ExternalInput")
    owt_d = nc.dram_tensor("out_wT", [nlayers, DI, D], F16, kind="ExternalInput")
    ng_d = nc.dram_tensor("norm_g", [nlayers, D], F32, kind="ExternalInput")
    nb_d = nc.dram_tensor("norm_b", [nlayers, D], F32, kind="ExternalInput")
    pwt_d = nc.dram_tensor("proj_wT", [2 * D, OUT], F32, kind="ExternalInput")
    pb_d = nc.dram_tensor("proj_b", [OUT], F32, kind="ExternalInput")
    ones1_d = nc.dram_tensor("ones1", [1, 128], F32, kind="ExternalInput")
    selc_d = nc.dram_tensor("selc", [DTR + 2 * S, 2 * S], F16, kind="ExternalInput")
    sel6c_d = nc.dram_tensor("sel6c", [6, 1], F32, kind="ExternalInput")

    out_d = nc.dram_tensor("out", [nb, OUT], F32, kind="ExternalOutput")
    dbg = {}
    if debug:
        for nm, sh in (("h0", [D, L_]), ("x1", [D, L_]), ("xi1", [DI, L_]),
                       ("dt1", [DI, L_]), ("y1", [DI, L_])):
            dbg[nm] = nc.dram_tensor("dbg_" + nm, sh, F32, kind="ExternalOutput")

    with tile.TileContext(nc) as tc:
        import contextlib
        with contextlib.ExitStack() as ctx:
            wpool = ctx.enter_context(tc.tile_pool(name="weights", bufs=1))
            wstr = ctx.enter_context(tc.tile_pool(name="wstream", bufs=1))
            apool = ctx.enter_context(tc.tile_pool(name="acts", bufs=1))
            spool = ctx.enter_context(tc.tile_pool(name="slab", bufs=1))
            s2pool = ctx.enter_context(tc.tile_pool(name="slab2", bufs=1))
            scpool = ctx.enter_context(tc.tile_pool(name="scan", bufs=1))
            rpool = ctx.enter_context(tc.tile_pool(name="rows", bufs=1))
            pp = ctx.enter_context(tc.tile_pool(name="ps_mm", bufs=1, space="PSUM"))
            pln = ctx.enter_context(tc.tile_pool(name="ps_ln", bufs=1, space="PSUM"))
            pbc = ctx.enter_context(tc.tile_pool(name="ps_bc", bufs=1, space="PSUM"))

            dma = nc.gpsimd.dma_start

            _wn = [0]

            def loadw(dram_ap, shape, dt=F32):
                _wn[0] += 1
                t = wpool.tile(shape, dt, name=f"w{_wn[0]}", tag=f"w{_wn[0]}")
                dma(out=t, in_=dram_ap)
                return t

            ones1 = loadw(ones1_d.ap(), [1, 128])
            selc = loadw(selc_d.ap(), [DTR + 2 * S, 2 * S], F16)
            sel6c = loadw(sel6c_d.ap(), [6, 1])

            def bc_stat(col):
                return bass.AP(tensor=selc.tensor, offset=selc.offset + col,
                               ap=[[2 * S, DTR + 2 * S], [0, 128]])
            onescol = wpool.tile([128, 1], F32)
            nc.vector.memset(onescol, 1.0)
            eps_t = wpool.tile([1, 1], F32)
            nc.vector.memset(eps_t, EPS)

            cwt = [loadw(cwt_d.ap()[:, m * 128:(m + 1) * 128], [5, 128]) for m in range(ND)]
            pe2 = loadw(pe2_d.ap().rearrange("(n p) l -> p n l", p=128), [128, ND, L_])

            def load_cols(dram_t, n, base):
                _wn[0] += 1
                t = wpool.tile([128, n], F32, name=f"w{_wn[0]}", tag=f"w{_wn[0]}")
                dma(out=t, in_=bass.AP(tensor=dram_t.ap().tensor, offset=base,
                                       ap=[[1, 128], [128, n]]))
                return t

            cont_b = load_cols(cb_d, ND, 0)
            ln_g = load_cols(lng_d, ND, 0)
            ln_b = load_cols(lnb_d, ND, 0)
            ddir = load_cols(ddir_d, ND, 0)
            pb_t = load_cols(pb_d, 1, 0)

            xpt = [[loadw(xpt_d.ap()[l, k * 128:(k + 1) * 128, :], [128, DTR + 2 * S], F16)
                    for k in range(NDI)] for l in range(nlayers)]
            dtwt = [loadw(dtwt_d.ap()[l], [DTR, DI], F16) for l in range(nlayers)]
            owt = [[loadw(owt_d.ap()[l, k * 128:(k + 1) * 128, :], [128, D], F16)
                    for k in range(NDI)] for l in range(nlayers)]
            pwt = [loadw(pwt_d.ap()[k * 128:(k + 1) * 128, :], [128, OUT])
                   for k in range(2 * ND)]

            def load_convw(l, m):
                _wn[0] += 1
                t = wpool.tile([128, K], F32, name=f"w{_wn[0]}", tag=f"w{_wn[0]}")
                dma(out=t, in_=bass.AP(tensor=cvw_d.ap().tensor,
                                       offset=(l * DI + m * 128) * K,
                                       ap=[[K, 128], [1, K]]))
                return t

            cvw = [[load_convw(l, m) for m in range(NDI)] for l in range(nlayers)]
            cvb = [load_cols(cvb_d, NDI, l * DI) for l in range(nlayers)]
            dtb = [load_cols(dtb_d, NDI, l * DI) for l in range(nlayers)]
            Dpw = [load_cols(Dp_d, NDI, l * DI) for l in range(nlayers)]
            ng = [load_cols(ng_d, ND, l * D) for l in range(nlayers)]
            nbt = [load_cols(nb_d, ND, l * D) for l in range(nlayers)]
            A_t = [[loadw(A_d.ap()[l, m * 128:(m + 1) * 128, :], [128, S])
                    for m in range(NDI)] for l in range(nlayers)]

            zcat = [apool.tile([128, nb], F32, tag=f"zcat{k}", name=f"zcat{k}") for k in range(2 * ND)]

            def layer_norm(x_aps, g_cols, b_cols, out_aps):
                ssum = pln.tile([1, TS], F32, tag="ln_sum")
                s2 = pln.tile([1, TS], F32, tag="ln_sum2")
                sqt = rpool.tile([128, TS], F32, tag="ln_sq")
                for i, xt in enumerate(x_aps):
                    nc.scalar.activation(out=sqt, in_=xt, func=AF.Square)
                    nc.tensor.matmul(s2, onescol, sqt,
                                     start=(i == 0), stop=(i == len(x_aps) - 1))
                for i, xt in enumerate(x_aps):
                    nc.tensor.matmul(ssum, onescol, xt,
                                     start=(i == 0), stop=(i == len(x_aps) - 1))
                murs = rpool.tile([1, 2 * TS], F32, tag="ln_murs")
                nc.scalar.activation(out=murs[:, 0:TS], in_=ssum, func=AF.Copy,
                                     scale=1.0 / D)
                r1 = rpool.tile([1, TS], F32, tag="ln_r1")
                nc.scalar.activation(out=r1, in_=s2, func=AF.Copy, scale=1.0 / D)
                r2 = rpool.tile([1, TS], F32, tag="ln_r2")
                nc.scalar.activation(out=r2, in_=murs[:, 0:TS], func=AF.Square)
                nc.vector.tensor_tensor(out=r1, in0=r1, in1=r2, op=OP.subtract)
                nc.scalar.activation(out=r1, in_=r1, func=AF.Sqrt,
                                     bias=eps_t[0:1, 0:1])
                nc.vector.reciprocal(out=murs[:, TS:], in_=r1)
                lnbc = pln.tile([128, TS], F32, tag="ln_bc")
                nc.tensor.matmul(lnbc, ones1, murs[:, 0:TS], start=True, stop=True)
                for i, xt in enumerate(x_aps):
                    eng = nc.gpsimd if (TUNE["ln_alt"] and i % 2) else nc.vector
                    eng.tensor_tensor(out=out_aps[i], in0=xt, in1=lnbc,
                                      op=OP.subtract)
                lnbc2 = pln.tile([128, TS], F32, tag="ln_bc")
                nc.tensor.matmul(lnbc2, ones1, murs[:, TS:], start=True, stop=True)
                for i in range(len(x_aps)):
                    eng = nc.gpsimd if (TUNE["ln_alt"] and i % 2) else nc.vector
                    eng.tensor_tensor(out=out_aps[i], in0=out_aps[i], in1=lnbc2,
                                      op=OP.mult)
                    eng.tensor_scalar(out=out_aps[i], in0=out_aps[i],
                                      scalar1=g_cols[:, i:i + 1],
                                      scalar2=b_cols[:, i:i + 1],
                                      op0=OP.mult, op1=OP.add)

            for b in range(nb):
                # ===== embedding =====
                xf16 = apool.tile([6, L_], F16, tag="xf16", bufs=2)
                dma(out=xf16, in_=xf_d.ap()[b])
                xf = apool.tile([6, L_], F32, tag="xf")
                nc.vector.tensor_copy(out=xf, in_=xf16)
                h_fwd = apool.tile([128, ND, L_], F32, tag="h_fwd", bufs=2)
                h_rev = apool.tile([128, ND, L_], F32, tag="h_rev", bufs=2)
                for islab in range(nslab):
                    t0, t1 = islab * TS, (islab + 1) * TS
                    e_sb = spool.tile([128, ND, TS], F32, tag="emb_e")
                    for m in range(ND):
                        ep = pp.tile([128, TS], F32, tag="mm_ps")
                        nc.tensor.matmul(ep, cwt[m], xf[0:5, t0:t1], start=True, stop=True)
                        nc.scalar.activation(out=e_sb[:, m, :], in_=ep, func=AF.Identity,
                                             bias=cont_b[:, m:m + 1])
                    xn = spool.tile([128, ND, TS], F32, tag="xn_e")
                    layer_norm([e_sb[:, m, :] for m in range(ND)], ln_g, ln_b,
                               [xn[:, m, :] for m in range(ND)])
                    mb = pp.tile([128, TS], F32, tag="mm_ps")
                    nc.tensor.matmul(
                        mb,
                        bass.AP(tensor=sel6c.tensor, offset=sel6c.offset,
                                ap=[[1, 6], [0, 128]]),
                        xf[:, t0:t1], start=True, stop=True)
                    for m in range(ND):
                        nc.scalar.activation(out=xn[:, m, :], in_=xn[:, m, :],
                                             func=AF.Gelu)
                        hm = h_fwd[:, m, t0:t1]
                        nc.vector.tensor_tensor(out=hm, in0=xn[:, m, :],
                                                in1=pe2[:, m, t0:t1], op=OP.add)
                        nc.vector.scalar_tensor_tensor(out=hm, in0=mb,
                                                       scalar=ddir[:, m:m + 1],
                                                       in1=hm, op0=OP.mult, op1=OP.add)
                for m in range(ND):
                    src = _ap(h_fwd, m * L_ + (L_ - 1), [h_fwd.ap[0], [-1, L_]])
                    nc.vector.tensor_copy(out=h_rev[:, m, :], in_=src)
                if debug and b == 0:
                    dma(out=dbg["h0"].ap().rearrange("(n p) l -> p n l", p=128), in_=h_fwd)

                # ===== mamba stacks =====
                for direction in range(2):
                    x_cur = h_fwd if direction == 0 else h_rev
                    lrange = (range(0, nlayers - nlayers // 2) if direction == 0
                              else range(nlayers - nlayers // 2, nlayers))
                    for li, l in enumerate(lrange):
                        inw = wstr.tile([128, ND, 2 * DI], F16, tag="inw")
                        dma(out=inw, in_=inwt_d.ap()[l].rearrange(
                            "(n p) e -> p n e", p=128))
                        if li == 0:
                            x_new = apool.tile([128, ND, L_], F32, tag="xnew0",
                                               bufs=2)
                        else:
                            x_new = h_fwd if direction == 0 else h_rev
                        carry = apool.tile([128, NDI, S], F16, tag="carry")
                        nc.vector.memset(carry, 0.0)
                        halo = apool.tile([128, NDI, K - 1], F16, tag="halo")
                        nc.vector.memset(halo, 0.0)
                        for islab in range(nslab):
                            t0, t1 = islab * TS, (islab + 1) * TS
                            xn = spool.tile([128, ND, TS], F16, tag="xn")
                            layer_norm([x_cur[:, m, t0:t1] for m in range(ND)],
                                       ng[l], nbt[l],
                                       [xn[:, m, :] for m in range(ND)])
                            xi_raw = spool.tile([128, NDI, K - 1 + TS], F16, tag="xi_raw")
                            z_t = spool.tile([128, NDI, TS], F16, tag="z")
                            xi_t = spool.tile([128, NDI, TS], F16, tag="xi")
                            dt_t = spool.tile([128, NDI, TS], F16, tag="dt")
                            y_t = spool.tile([128, NDI, TS], F16, tag="y")
                            nc.vector.tensor_copy(
                                out=_ap(xi_raw, 0,
                                        [xi_raw.ap[0], [K - 1 + TS, NDI], [1, K - 1]]),
                                in_=halo)
                            for m in range(2 * NDI):
                                psm = pp.tile([128, TS], F32, tag="mm_ps")
                                for k in range(ND):
                                    nc.tensor.matmul(psm, inw[:, k, m * 128:(m + 1) * 128],
                                                     xn[:, k, :], start=(k == 0),
                                                     stop=(k == ND - 1))
                                if m < NDI:
                                    nc.scalar.activation(out=xi_raw[:, m, K - 1:],
                                                         in_=psm, func=AF.Copy)
                                else:
                                    # silu fused into the evacuation
                                    nc.scalar.activation(out=z_t[:, m - NDI, :],
                                                         in_=psm, func=AF.Silu)
                            nc.vector.tensor_copy(
                                out=halo,
                                in_=_ap(xi_raw, TS,
                                        [xi_raw.ap[0], [K - 1 + TS, NDI], [1, K - 1]]))
                            # conv + silu (z already silu'd at evac)
                            for m in range(NDI):
                                ceng = (nc.gpsimd if (TUNE["conv_alt"] and m % 2)
                                        else nc.vector)
                                acc = s2pool.tile([128, TS], F16, tag="convacc")
                                ceng.tensor_scalar(out=acc, in0=xi_raw[:, m, K - 1:],
                                                   scalar1=cvw[l][m][:, K - 1:K],
                                                   scalar2=None, op0=OP.mult)
                                for kk in range(K - 2, -1, -1):
                                    ceng.scalar_tensor_tensor(
                                        out=acc, in0=xi_raw[:, m, kk:kk + TS],
                                        scalar=cvw[l][m][:, kk:kk + 1],
                                        in1=acc, op0=OP.mult, op1=OP.add)
                                nc.scalar.activation(out=xi_t[:, m, :], in_=acc,
                                                     func=AF.Silu, bias=cvb[l][:, m:m + 1])
                            # xproj
                            xdb_ps = pp.tile([DTR + 2 * S, TS], F32, tag="mm_ps")
                            for k in range(NDI):
                                nc.tensor.matmul(xdb_ps, xpt[l][k], xi_t[:, k, :],
                                                 start=(k == 0), stop=(k == NDI - 1))
                            xdb = s2pool.tile([DTR + 2 * S, TS], F16, tag="xdb")
                            nc.scalar.activation(out=xdb, in_=xdb_ps, func=AF.Copy)
                            # dt proj + softplus (two act passes: all Exp,
                            # then all Ln, avoiding per-m table ping-pong); dtu
                            spx = s2pool.tile([128, NDI, TS], F32, tag="spx")
                            for m in range(NDI):
                                dps = pp.tile([128, TS], F32, tag="mm_ps")
                                nc.tensor.matmul(dps, dtwt[l][:, m * 128:(m + 1) * 128],
                                                 xdb[0:DTR, :], start=True, stop=True)
                                nc.scalar.activation(out=spx[:, m, :], in_=dps,
                                                     func=AF.Exp,
                                                     bias=dtb[l][:, m:m + 1])
                            nc.scalar.activation(out=dt_t, in_=spx, func=AF.Ln,
                                                 bias=onescol[:, 0:1])
                            for m in range(NDI):
                                deng = (nc.gpsimd if (TUNE["dt_alt"] and m % 2)
                                        else nc.vector)
                                deng.tensor_scalar(out=y_t[:, m, :],
                                                   in0=xi_t[:, m, :],
                                                   scalar1=Dpw[l][:, m:m + 1],
                                                   scalar2=None, op0=OP.mult)
                                deng.tensor_tensor(out=xi_t[:, m, :],
                                                   in0=xi_t[:, m, :],
                                                   in1=dt_t[:, m, :], op=OP.mult)
                            # scan over s-groups: B/C matmuls into one 4-bank
                            # PSUM tile, single Act evac to f16 SBUF per group
                            for g in range(NSG):
                                BCps = pbc.tile([128, 2 * SG, TS], F32, tag="BCps")
                                for j in range(SG):
                                    s = g * SG + j
                                    nc.tensor.matmul(BCps[:, j, :], bc_stat(s),
                                                     xdb, start=True, stop=True)
                                    nc.tensor.matmul(BCps[:, SG + j, :],
                                                     bc_stat(S + s),
                                                     xdb, start=True, stop=True)
                                BCb = scpool.tile([128, 2 * SG, TS], F16, tag="BCb",
                                                  bufs=2)
                                nc.scalar.activation(out=BCb, in_=BCps, func=AF.Copy)
                                for m in range(NDI):
                                    seng = nc.vector
                                    beng = (nc.gpsimd if (TUNE["bp_alt"] and m % 2)
                                            else nc.vector)
                                    peng = (nc.gpsimd if (TUNE["bp_alt"] and m % 2 == 0)
                                            else nc.vector)
                                    aux = nc.gpsimd if TUNE["aux_pool"] else nc.vector
                                    a_t = scpool.tile([128, SG, TS + 1], F16, tag="a_t", bufs=2)
                                    b_t = scpool.tile([128, SG, TS + 1], F16, tag="b_t", bufs=2)
                                    h_t = scpool.tile([128, SG, TS + 1], F16, tag="h_t", bufs=2)
                                    for j in range(SG):
                                        s = g * SG + j
                                        nc.scalar.activation(out=a_t[:, j, 1:],
                                                             in_=dt_t[:, m, :],
                                                             func=AF.Exp,
                                                             scale=A_t[l][m][:, s:s + 1])
                                    aux.memset(
                                        _ap(a_t, 0, [a_t.ap[0], [TS + 1, SG], [1, 1]]), 0.0)
                                    aux.tensor_copy(
                                        out=_ap(b_t, 0, [b_t.ap[0], [TS + 1, SG], [1, 1]]),
                                        in_=_ap(carry, m * S + g * SG,
                                                [carry.ap[0], [1, SG], [1, 1]]))
                                    dtu_rep = _ap(xi_t, m * TS,
                                                  [xi_t.ap[0], [0, SG], [1, TS]])
                                    beng.tensor_tensor(
                                        out=_ap(b_t, 1, [b_t.ap[0], [TS + 1, SG], [1, TS]]),
                                        in0=dtu_rep, in1=BCb[:, 0:SG, :], op=OP.mult)
                                    seng.tensor_tensor_scan(
                                        out=_ap(h_t, 0, [h_t.ap[0], [1, SG * (TS + 1)]]),
                                        data0=_ap(a_t, 0, [a_t.ap[0], [1, SG * (TS + 1)]]),
                                        data1=_ap(b_t, 0, [b_t.ap[0], [1, SG * (TS + 1)]]),
                                        initial=0.0, op0=OP.mult, op1=OP.add)
                                    aux.tensor_copy(
                                        out=_ap(carry, m * S + g * SG,
                                                [carry.ap[0], [1, SG], [1, 1]]),
                                        in_=_ap(h_t, TS, [h_t.ap[0], [TS + 1, SG], [1, 1]]))
                                    p_t = scpool.tile([128, SG, TS], F16, tag="p_t")
                                    peng.tensor_tensor(
                                        out=p_t,
                                        in0=_ap(h_t, 1, [h_t.ap[0], [TS + 1, SG], [1, TS]]),
                                        in1=BCb[:, SG:2 * SG, :], op=OP.mult)
                                    yg = s2pool.tile([128, TS], F16, tag="yg")
                                    nc.vector.tensor_tensor(out=yg, in0=p_t[:, 0, :],
                                                            in1=p_t[:, 1, :], op=OP.add)
                                    nc.vector.tensor_tensor(out=y_t[:, m, :],
                                                            in0=y_t[:, m, :],
                                                            in1=yg, op=OP.add)
                            # gate (z already silu'd at evac)
                            for m in range(NDI):
                                geng = (nc.gpsimd if (TUNE["gate_alt"] and m % 2)
                                        else nc.vector)
                                geng.tensor_tensor(out=y_t[:, m, :], in0=y_t[:, m, :],
                                                   in1=z_t[:, m, :], op=OP.mult)
                            # out_proj + residual
                            for m in range(ND):
                                ops = pp.tile([128, TS], F32, tag="mm_ps")
                                for k in range(NDI):
                                    nc.tensor.matmul(ops, owt[l][k][:, m * 128:(m + 1) * 128],
                                                     y_t[:, k, :], start=(k == 0),
                                                     stop=(k == NDI - 1))
                                oeng = (nc.gpsimd if (TUNE["out_alt"] and m % 2)
                                        else nc.vector)
                                oeng.tensor_tensor(out=x_new[:, m, t0:t1],
                                                   in0=x_cur[:, m, t0:t1],
                                                   in1=ops, op=OP.add)
                            if debug and b == 0 and l == 0:
                                for m in range(NDI):
                                    dma(out=dbg["xi1"].ap().rearrange(
                                        "(n p) l -> p n l", p=128)[:, m, t0:t1],
                                        in_=xi_t[:, m, :])
                                    dma(out=dbg["dt1"].ap().rearrange(
                                        "(n p) l -> p n l", p=128)[:, m, t0:t1],
                                        in_=dt_t[:, m, :])
                                    dma(out=dbg["y1"].ap().rearrange(
                                        "(n p) l -> p n l", p=128)[:, m, t0:t1],
                                        in_=y_t[:, m, :])
                        x_cur = x_new
                        if debug and b == 0 and l == 0:
                            dma(out=dbg["x1"].ap().rearrange("(n p) l -> p n l", p=128),
                                in_=x_cur)
                    for m in range(ND):
                        mean = rpool.tile([128, 1], F32, tag="mean")
                        nc.vector.tensor_reduce(out=mean, in_=x_cur[:, m, :],
                                                axis=mybir.AxisListType.X, op=OP.add)
                        nc.scalar.activation(out=zcat[direction * ND + m][:, b:b + 1],
                                             in_=mean, func=AF.Copy, scale=1.0 / L_)

            prj = pp.tile([OUT, nb], F32, tag="mm_ps")
            for k in range(2 * ND):
                nc.tensor.matmul(prj, pwt[k], zcat[k], start=(k == 0),
                                 stop=(k == 2 * ND - 1))
            ob = rpool.tile([OUT, nb], F32, tag="out_sb")
            nc.scalar.activation(out=ob, in_=prj, func=AF.Identity, bias=pb_t[:, 0:1])
            dma(out=bass.AP(tensor=out_d.ap().tensor, offset=0,
                            ap=[[1, OUT], [OUT, nb]]), in_=ob)
    nc.compile()
    return nc


_cache = {}


def _prep_common(inputs, nlayers=NL, L_=L):
    import math
    pos = np.arange(L_, dtype=np.float32)[:, None]
    div = np.exp(np.arange(0, D, 2, dtype=np.float32) * (-math.log(10000.0) / D))
    pe = np.zeros((L_, D), np.float32)
    pe[:, 0::2] = np.sin(pos * div)
    pe[:, 1::2] = np.cos(pos * div)
    dir_emb = np.asarray(inputs["dir_emb"], np.float32)
    pe2 = np.ascontiguousarray((pe + dir_emb[0][None, :]).T)

    common = dict(
        pe2=pe2,
        ddir=np.ascontiguousarray(dir_emb[1] - dir_emb[0]),
        cont_wT=np.ascontiguousarray(np.asarray(inputs["cont_w"], np.float32).T),
        cont_b=np.asarray(inputs["cont_b"], np.float32),
        ln_g=np.asarray(inputs["ln_g"], np.float32),
        ln_b=np.asarray(inputs["ln_b"], np.float32),
        in_wT=np.ascontiguousarray(
            np.asarray(inputs["in_w"], np.float32)[:nlayers].transpose(0, 2, 1)).astype(np.float16),
        conv_w=np.ascontiguousarray(
            np.asarray(inputs["conv_w"], np.float32)[:nlayers, :, 0, :]),
        conv_b=np.asarray(inputs["conv_b"], np.float32)[:nlayers],
        xproj_wT=np.ascontiguousarray(
            np.asarray(inputs["xproj_w"], np.float32)[:nlayers].transpose(0, 2, 1)).astype(np.float16),
        dt_wT=np.ascontiguousarray(
            np.asarray(inputs["dt_w"], np.float32)[:nlayers].transpose(0, 2, 1)).astype(np.float16),
        dt_b=np.asarray(inputs["dt_b"], np.float32)[:nlayers],
        A=np.ascontiguousarray(
            -np.exp(np.asarray(inputs["A_log"], np.float32)[:nlayers])),
        Dp=np.asarray(inputs["Dp"], np.float32)[:nlayers],
        out_wT=np.ascontiguousarray(
            np.asarray(inputs["out_w"], np.float32)[:nlayers].transpose(0, 2, 1)).astype(np.float16),
        norm_g=np.asarray(inputs["norm_g"], np.float32)[:nlayers],
        norm_b=np.asarray(inputs["norm_b"], np.float32)[:nlayers],
        proj_wT=np.ascontiguousarray(np.asarray(inputs["proj_w"], np.float32).T),
        proj_b=np.asarray(inputs["proj_b"], np.float32),
    )
    common["ones1"] = np.ones((1, 128), np.float32)
    selc = np.zeros((DTR + 2 * S, 2 * S), np.float16)
    for i in range(2 * S):
        selc[DTR + i, i] = 1.0
    sel6c = np.zeros((6, 1), np.float32)
    sel6c[5, 0] = 1.0
    common["selc"] = selc
    common["sel6c"] = sel6c
    return common


def _prep_xf(inputs, L_=L):
    x = np.asarray(inputs["x"], np.float32)
    cont_idx = [0, 1, 3, 4, 5]
    xs = x[:, :L_]
    xf = np.empty((B, 6, L_), np.float16)
    xf[:, 0:5, :] = xs[..., cont_idx].transpose(0, 2, 1)
    xf[:, 5, :] = (xs[:, :, 2] > 0).astype(np.float16)
    return xf


# weight-bearing inputs whose values feed _prep_common (everything but x)
_WKEYS = ("cont_w", "cont_b", "ln_g", "ln_b", "dir_emb", "in_w", "conv_w",
          "conv_b", "xproj_w", "dt_w", "dt_b", "A_log", "Dp", "out_w",
          "norm_g", "norm_b", "proj_w", "proj_b")


def _make_runner(nc, ncores=NCORES):
    """Persistent jit over the bass module, mirroring bass2jax.run_bass_via_pjrt
    but built once so steady-state calls skip retrace/recompile/NEFF reload."""
    import jax
    from jax.experimental.shard_map import shard_map
    from jax.sharding import Mesh, PartitionSpec, NamedSharding
    from concourse import bass2jax
    from concourse.bass2jax import _bass_exec_p, partition_id_tensor

    bass2jax.install_neuronx_cc_hook()

    partition_name = nc.partition_id_tensor.name if nc.partition_id_tensor else None
    in_names, out_names, out_avals, zero_shapes = [], [], [], []
    for alloc in nc.m.functions[0].allocations:
        if not isinstance(alloc, mybir.MemoryLocationSet):
            continue
        name = alloc.memorylocations[0].name
        if alloc.kind == "ExternalInput":
            if name != partition_name:
                in_names.append(name)
        elif alloc.kind == "ExternalOutput":
            out_names.append(name)
            shape = tuple(alloc.tensor_shape)
            dtype = mybir.dt.np(alloc.dtype)
            out_avals.append(jax.core.ShapedArray(shape, dtype))
            zero_shapes.append(((ncores * shape[0], *shape[1:]), dtype))
    n_params = len(in_names)
    n_outs = len(out_names)
    all_in = list(in_names) + list(out_names)
    if partition_name is not None:
        all_in.append(partition_name)

    def _body(*args):
        operands = list(args)
        if partition_name is not None:
            operands.append(partition_id_tensor())
        outs = _bass_exec_p.bind(
            *operands,
            out_avals=tuple(out_avals),
            in_names=tuple(all_in),
            out_names=tuple(out_names),
            lowering_input_output_aliases=(),
            sim_require_finite=True,
            sim_require_nnan=True,
            nc=nc,
        )
        return tuple(outs)

    devices = jax.devices()[:ncores]
    mesh = Mesh(np.asarray(devices), ("core",))
    P = PartitionSpec
    jitfn = jax.jit(
        shard_map(_body, mesh=mesh,
                  in_specs=(P("core"),) * (n_params + n_outs),
                  out_specs=(P("core"),) * n_outs,
                  check_rep=False),
        donate_argnums=tuple(range(n_params, n_params + n_outs)),
        keep_unused=True)
    sharding = NamedSharding(mesh, P("core"))
    return dict(jitfn=jitfn, in_names=in_names, out_names=out_names,
                zero_shapes=zero_shapes, sharding=sharding, jax=jax)


def _weights_current(inputs):
    cached = _cache.get("wraw")
    if cached is None:
        return False
    for k in _WKEYS:
        a, b = cached[k], inputs[k]
        if a is b:
            continue
        if not np.array_equal(a, np.asarray(b)):
            return False
    return True


def kernel(**inputs):
    if bool(int(os.environ.get("KERNEL_TRACE", "0"))):
        return _kernel_traced(**inputs)
    if "nc" not in _cache:
        _cache["nc"] = build()
        _cache["runner"] = _make_runner(_cache["nc"])
    run = _cache["runner"]
    jax = run["jax"]

    if not _weights_current(inputs):
        common = _prep_common(inputs)
        dev = {}
        for name in run["in_names"]:
            if name == "xf":
                continue
            w = common[name]
            rep = np.concatenate([w] * NCORES, axis=0)
            dev[name] = jax.device_put(rep, run["sharding"])
        _cache["wdev"] = dev
        _cache["wraw"] = {k: np.asarray(inputs[k]) for k in _WKEYS}

    xf = _prep_xf(inputs)
    wdev = _cache["wdev"]
    args = [xf if name == "xf" else wdev[name] for name in run["in_names"]]
    args += [np.zeros(sh, dt) for sh, dt in run["zero_shapes"]]
    outs = run["jitfn"](*args)
    oidx = run["out_names"].index("out")
    out = np.asarray(outs[oidx])          # (B, OUT), batch-concat across cores
    return np.ascontiguousarray(out.astype(np.float32))


def _kernel_traced(**inputs):
    """Profiling path: one-shot run through run_bass_kernel_spmd with trace."""
    if "nc" not in _cache:
        _cache["nc"] = build()
    nc = _cache["nc"]
    common = _prep_common(inputs)
    xf = _prep_xf(inputs)
    in_maps = []
    for c in range(NCORES):
        m = dict(common)
        m["xf"] = np.ascontiguousarray(xf[c * NB:(c + 1) * NB])
        in_maps.append(m)
    res = run_bass_kernel_spmd(nc, in_maps, core_ids=list(range(NCORES)),
                               trace=True)
    _cache["last_result"] = res
    out = np.concatenate([res.results[c]["out"] for c in range(NCORES)], axis=0)
    return np.ascontiguousarray(out.astype(np.float32))

